# revision 10
# baseline (speedup 1.0000x reference)
"""Trainium2 Bass kernel for nn_CustomEncoderBlock (dense transformer encoder).

Sharding: pure data parallel over batch (64) across 8 NeuronCores, params
replicated. Token-major activations; PE-transposes feed f32r matmuls.
Pool attention is algebraically folded: since q_len==1 and the pool query is
input-independent, the packed k-projection collapses into U @ rms(feat) and
the v-projection is applied after pooling (per-head weighted feature sums).
Encoder attention runs over all 88 packed tokens with a block-diagonal mask.
"""
import numpy as np

import concourse.bass as bass
import concourse.tile as tile
import concourse.mybir as mybir
from concourse import bacc
from concourse.bass_utils import run_bass_kernel_spmd
from concourse.masks import make_identity

F32 = mybir.dt.float32
F32R = mybir.dt.float32r
I32 = mybir.dt.int32
ADD = mybir.AluOpType.add
SUB = mybir.AluOpType.subtract
MUL = mybir.AluOpType.mult
MAX = mybir.AluOpType.max
AX = mybir.AxisListType.X
AF = mybir.ActivationFunctionType

NCORES = 8
B, TO, S, KVL = 64, 8, 512, 1536
D, H, HD = 1024, 16, 64
BL = B // NCORES
T = 3 + TO
NT = BL * T                      # 88 tokens per core
DFF = 4 * D
NL = 4
P = 128
KO = KVL // P                    # 12
DO = D // P                      # 8
TWO_PI = float(2 * np.pi)
HALF_PI = float(np.pi / 2)

_CACHE = {}


def build_nc():
    nc = bacc.Bacc(None, target_bir_lowering=False)

    def din(name, shape, dtype=F32):
        return nc.dram_tensor(name, list(shape), dtype, kind="ExternalInput")

    x_vl = din("x_vl", (BL * S, KVL))
    x_r = din("x_r", (BL * S, KVL))
    x_cond = din("x_cond", (BL * TO, 512))
    tvec = din("tvec", (BL, 1))
    freqs = din("freqs", (1, 512))
    enc_mask = din("enc_mask", (NT, NT))

    per_pref = {}
    for pref in ("vl", "r"):
        per_pref[pref] = dict(
            Wt=din(f"Wt_{pref}", (KVL, D), F32R),
            pb=din(f"pb_{pref}", (1, D)),
            UT=din(f"UT_{pref}", (D, H), F32R),
            wvT=din(f"wvT_{pref}", (D, D), F32R),
            bv=din(f"bv_{pref}", (1, D)),
            WoT=din(f"WoT_{pref}", (D, D), F32R),
            ob=din(f"ob_{pref}", (1, D)),
        )
    WtC = din("WtC", (512, D), F32R)
    pbC = din("pbC", (1, D))

    layers = []
    for i in range(NL):
        layers.append(dict(
            WinT=din(f"L{i}_WinT", (D, 3 * D), F32R),
            inb=din(f"L{i}_inb", (1, 3 * D)),
            WoutT=din(f"L{i}_WoutT", (D, D), F32R),
            outb=din(f"L{i}_outb", (1, D)),
            W1T=din(f"L{i}_W1T", (D, DFF), F32R),
            b1=din(f"L{i}_b1", (1, DFF)),
            W2T=din(f"L{i}_W2T", (DFF, D), F32R),
            b2=din(f"L{i}_b2", (1, D)),
        ))
    mp1T = din("mp1T", (D, D), F32R)
    mp1b = din("mp1b", (1, D))
    mp2T = din("mp2T", (D, D), F32R)
    mp2b = din("mp2b", (1, D))

    y_out = nc.dram_tensor("y", [NT, D], F32, kind="ExternalOutput")

    with tile.TileContext(nc) as tc:
        with tc.tile_pool(name="const", bufs=1) as const, \
             tc.tile_pool(name="sb_keep", bufs=1) as sb_keep, \
             tc.tile_pool(name="sb_small", bufs=1) as sb_small, \
             tc.tile_pool(name="sb_scr", bufs=1) as sb_scr, \
             tc.tile_pool(name="sb_stream", bufs=2) as sb_stream, \
             tc.tile_pool(name="ps_mm", bufs=3, space="PSUM") as ps_mm, \
             tc.tile_pool(name="ps_t", bufs=2, space="PSUM") as ps_t, \
             tc.tile_pool(name="ps_s", bufs=2, space="PSUM") as ps_s, \
             tc.tile_pool(name="dram", bufs=1, space="DRAM") as dram:

            ident = const.tile([P, P], F32)
            make_identity(nc, ident)
            eps5 = const.tile([P, 1], F32)
            nc.vector.memset(eps5, 1e-5)
            eps6 = const.tile([P, 1], F32)
            nc.vector.memset(eps6, 1e-6)

            def t_psum():
                return ps_t.tile([P, 512], F32, tag="t_ps", name="t_ps")

            def rep_vec(dram_vec, n, tag, pool=None, col0=0):
                """[1,n] slice of a DRAM vector -> [128,n] broadcast f32 tile."""
                rep = (pool or sb_keep).tile([P, n], F32, tag="rep_" + tag,
                                             name="rep_" + tag)
                src = dram_vec[:, col0:col0 + n]
                bc = bass.AP(tensor=src.tensor, offset=src.offset,
                             ap=[[0, P], [1, n]])
                nc.gpsimd.dma_start(out=rep, in_=bc)
                return rep

            def transpose_into(src_fn, dst, dst_j, rows, kblocks, dcol=None,
                               scale=None):
                for kb in range(kblocks):
                    pt = t_psum()
                    nc.tensor.transpose(pt[:, :rows], src_fn(kb),
                                        ident[:rows, :rows])
                    col = kb if dcol is None else dcol
                    dsl = dst[:, col, dst_j * rows:(dst_j + 1) * rows]
                    if scale is None:
                        nc.vector.tensor_copy(out=dsl, in_=pt[:, :rows])
                    else:
                        nc.vector.tensor_scalar(out=dsl, in0=pt[:, :rows],
                                                scalar1=scale, scalar2=None,
                                                op0=MUL)

            def layer_norm(rows, x_ap, width, eps_tile):
                ssum = sb_small.tile([P, 1], F32, tag="ln_sum", name="ln_sum")
                nc.vector.tensor_reduce(out=ssum[:rows], in_=x_ap, axis=AX, op=ADD)
                sq = sb_scr.tile([P, 1024], F32, tag="sq1024", name="sq")
                ssq = sb_small.tile([P, 1], F32, tag="ln_ssq", name="ln_ssq")
                nc.scalar.activation(sq[:rows, :width], x_ap, AF.Square,
                                     accum_out=ssq[:rows])
                nmean = sb_small.tile([P, 1], F32, tag="ln_nm", name="ln_nm")
                nc.vector.tensor_scalar(out=nmean[:rows], in0=ssum[:rows],
                                        scalar1=-1.0 / width, scalar2=None, op0=MUL)
                m2 = sb_small.tile([P, 1], F32, tag="ln_m2", name="ln_m2")
                nc.vector.tensor_tensor(out=m2[:rows], in0=nmean[:rows],
                                        in1=nmean[:rows], op=MUL)
                var = sb_small.tile([P, 1], F32, tag="ln_var", name="ln_var")
                nc.vector.tensor_scalar(out=var[:rows], in0=ssq[:rows],
                                        scalar1=1.0 / width, scalar2=m2[:rows],
                                        op0=MUL, op1=SUB)
                nc.scalar.activation(var[:rows], var[:rows], AF.Sqrt,
                                     bias=eps_tile[:rows])
                nc.vector.reciprocal(out=var[:rows], in_=var[:rows])
                nc.vector.tensor_scalar(out=x_ap, in0=x_ap, scalar1=nmean[:rows],
                                        scalar2=var[:rows], op0=ADD, op1=MUL)

            def stream_unit(wdram, k0, n0, tag="wunit"):
                u = sb_stream.tile([P, DO, 512], F32R, tag=tag, name=tag)
                nc.sync.dma_start(
                    u, wdram[k0:k0 + D, n0:n0 + 512]
                    .rearrange("(c p) n -> p c n", p=P))
                return u

            # ======== t_emb ========
            te = sb_keep.tile([BL, D], F32, tag="te")
            with tc.tile_pool(name="temb", bufs=1) as temb:
                fr_rep = temb.tile([BL, 512], F32, tag="fr_rep")
                fsrc = freqs[:, :]
                nc.gpsimd.dma_start(out=fr_rep, in_=bass.AP(
                    tensor=fsrc.tensor, offset=fsrc.offset, ap=[[0, BL], [1, 512]]))
                t_sb = temb.tile([BL, 1], F32, tag="tvec")
                nc.sync.dma_start(t_sb, tvec[:, :])
                ang = temb.tile([BL, 512], F32, tag="ang")
                nc.vector.tensor_scalar_mul(ang, fr_rep, t_sb)
                for half, shift in ((0, 0.0), (1, HALF_PI)):
                    a2 = temb.tile([BL, 512], F32, tag="a2")
                    nc.vector.tensor_scalar(out=a2, in0=ang, scalar1=shift,
                                            scalar2=1.0 / TWO_PI, op0=ADD, op1=MUL)
                    mi = temb.tile([BL, 512], I32, tag="mi")
                    nc.vector.tensor_copy(out=mi, in_=a2)
                    mf = temb.tile([BL, 512], F32, tag="mf")
                    nc.vector.tensor_copy(out=mf, in_=mi)
                    nc.vector.tensor_tensor(out=mf, in0=a2, in1=mf, op=SUB)
                    nc.vector.tensor_scalar(out=mf, in0=mf, scalar1=TWO_PI,
                                            scalar2=None, op0=MUL)
                    nc.scalar.activation(te[:, half * 512:(half + 1) * 512], mf,
                                         AF.Sin)

            # ======== stage A ========
            pooled = {}
            with tc.tile_pool(name="sa_w", bufs=1) as sa_w, \
                 tc.tile_pool(name="sa_keep", bufs=1) as sa_keep, \
                 tc.tile_pool(name="sa_feat", bufs=1) as sa_feat, \
                 tc.tile_pool(name="sa_x", bufs=2) as sa_x:

                # ---- cond projection ----
                cond_e = sb_keep.tile([BL * TO, D], F32, tag="cond_e")
                with tc.tile_pool(name="sa_cond", bufs=1) as sa_cond:
                    WtC_sb = sa_cond.tile([P, 4, D], F32R, tag="WtC")
                    nc.sync.dma_start(WtC_sb,
                                      WtC[:, :].rearrange("(c p) n -> p c n", p=P))
                    pbC_rep = rep_vec(pbC, D, "pbC", pool=sa_cond)
                    xc = sa_x.tile([BL * TO, 512], F32, tag="xc")
                    nc.sync.dma_start(xc, x_cond[:, :])
                    condT = sa_cond.tile([P, 4, BL * TO], F32R, tag="condT")
                    transpose_into(lambda kb: xc[:, kb * P:(kb + 1) * P], condT,
                                   0, BL * TO, 4)
                    for n in range(2):
                        pm = ps_mm.tile([P, 512], F32, tag="mm_ps", name="pm")
                        for j in range(4):
                            nc.tensor.matmul(
                                pm[:BL * TO], lhsT=condT[:, j, :],
                                rhs=WtC_sb[:, j, n * 512:(n + 1) * 512],
                                start=(j == 0), stop=(j == 3))
                        nc.vector.tensor_tensor(
                            out=cond_e[:, n * 512:(n + 1) * 512],
                            in0=pm[:BL * TO],
                            in1=pbC_rep[:BL * TO, n * 512:(n + 1) * 512], op=ADD)

                # ---- per-modality projection + pool ----
                for pref, xin in (("vl", x_vl), ("r", x_r)):
                    pw = per_pref[pref]
                    W_sb = sa_w.tile([P, KO, D], F32R, tag="Wbig")
                    nc.sync.dma_start(
                        W_sb, pw["Wt"][:, :].rearrange("(c p) n -> p c n", p=P))
                    pb_rep = rep_vec(pw["pb"], D, "pb", pool=sa_keep)
                    UT_sb = sa_w.tile([P, DO, H], F32R, tag="UT")
                    nc.sync.dma_start(
                        UT_sb, pw["UT"][:, :].rearrange("(c p) n -> p c n", p=P))
                    bv_rep = rep_vec(pw["bv"], D, "bv", pool=sa_keep)
                    ob_rep = rep_vec(pw["ob"], D, "ob", pool=sa_keep)

                    GT = sa_keep.tile([P, DO, H, BL], F32R, tag="GT")
                    xin_v = xin[:, :].rearrange("(b s) k -> b s k", s=S)

                    for b in range(BL):
                        feat = sa_feat.tile([P, S // P, D], F32R, tag="feat")
                        for i in range(S // P):
                            x_in = sa_x.tile([P, KVL], F32, tag="x_in")
                            nc.sync.dma_start(x_in, xin_v[b, i * P:(i + 1) * P, :])
                            xT_c = sa_x.tile([P, KO, P], F32R, tag="xT_c")
                            transpose_into(
                                lambda kb: x_in[:, kb * P:(kb + 1) * P],
                                xT_c, 0, P, KO)
                            for n in range(2):
                                pm = ps_mm.tile([P, 512], F32, tag="mm_ps",
                                                name="pm")
                                for j in range(KO):
                                    nc.tensor.matmul(
                                        pm, lhsT=xT_c[:, j, :],
                                        rhs=W_sb[:, j, n * 512:(n + 1) * 512],
                                        start=(j == 0), stop=(j == KO - 1))
                                nc.vector.tensor_tensor(
                                    out=feat[:, i, n * 512:(n + 1) * 512],
                                    in0=pm, in1=pb_rep[:, n * 512:(n + 1) * 512],
                                    op=ADD)
                            layer_norm(P, feat[:, i, :], D, eps5)

                        # per-head rms rstd [128, 4, 16]
                        rstd = sb_small.tile([P, S // P, H], F32, tag="rstd")
                        for i in range(S // P):
                            sq = sb_scr.tile([P, 1024], F32, tag="sq1024",
                                             name="sq")
                            nc.scalar.activation(sq, feat[:, i, :], AF.Square)
                            nc.vector.tensor_reduce(
                                out=rstd[:, i, :],
                                in_=sq[:].rearrange("p (h d) -> p h d", d=HD),
                                axis=AX, op=ADD)
                        rsf = rstd[:].rearrange("p a b -> p (a b)")
                        nc.scalar.activation(rsf, rsf, AF.Sqrt, bias=eps6,
                                             scale=1.0 / HD)
                        nc.vector.reciprocal(out=rsf, in_=rsf)

                        # kn per s-chunk -> knT_i [128, 8, 128] f32r -> scores
                        psc = ps_s.tile([P, 512], F32, tag="s_ps", name="psc")
                        for i in range(S // P):
                            knT_i = sa_x.tile([P, DO, P], F32R, tag="knT_i")
                            for m in range(DO):
                                knb = sb_scr.tile([P, P], F32, tag="knb",
                                                  name="knb")
                                nc.vector.tensor_tensor(
                                    out=knb[:].rearrange("p (a b) -> p a b", b=HD),
                                    in0=feat[:, i, m * P:(m + 1) * P].rearrange(
                                        "p (a b) -> p a b", b=HD),
                                    in1=rstd[:, i, 2 * m:2 * m + 2, None]
                                    .to_broadcast((P, 2, HD)),
                                    op=MUL)
                                pt2 = t_psum()
                                nc.tensor.transpose(pt2[:, :P], knb, ident)
                                nc.vector.tensor_copy(out=knT_i[:, m, :],
                                                      in_=pt2[:, :P])
                            for m in range(DO):
                                nc.tensor.matmul(
                                    psc[:H, i * P:(i + 1) * P],
                                    lhsT=UT_sb[:, m, :], rhs=knT_i[:, m, :],
                                    start=(m == 0), stop=(m == DO - 1))
                        nmax = sb_small.tile([H, 1], F32, tag="nmax")
                        nc.vector.tensor_reduce(out=nmax, in_=psc[:H], axis=AX,
                                                op=MAX, negate=True)
                        attn = sb_small.tile([H, S], F32, tag="attn")
                        den = sb_small.tile([H, 1], F32, tag="den")
                        nc.scalar.activation(attn, psc[:H], AF.Exp, bias=nmax,
                                             accum_out=den)
                        nc.vector.reciprocal(out=den, in_=den)
                        nc.vector.tensor_scalar_mul(attn, attn, den)
                        attnT = sb_small.tile([P, S // P, H], F32R, tag="attnT")
                        transpose_into(lambda kb: attn[:, kb * P:(kb + 1) * P],
                                       attnT, 0, H, S // P)
                        for m in range(DO):
                            pg = ps_s.tile([P, 512], F32, tag="s_ps", name="pg")
                            for i in range(S // P):
                                nc.tensor.matmul(
                                    pg[:, :H],
                                    lhsT=feat[:, i, m * P:(m + 1) * P],
                                    rhs=attnT[:, i, :],
                                    start=(i == 0), stop=(i == S // P - 1))
                            nc.vector.tensor_copy(out=GT[:, m, :, b],
                                                  in_=pg[:, :H])

                    # ---- apply wv per head, then out-proj ----
                    O_sb = sb_small.tile([BL, D], F32, tag="O_sb")
                    for half in range(2):
                        po = ps_mm.tile([P, 512], F32, tag="mm_ps", name="po")
                        wu = stream_unit(pw["wvT"], 0, half * 512)
                        for h8 in range(DO):
                            h = half * DO + h8
                            for k in range(DO):
                                nc.tensor.matmul(
                                    po[:BL, h8 * HD:(h8 + 1) * HD],
                                    lhsT=GT[:, k, h, :],
                                    rhs=wu[:, k, h8 * HD:(h8 + 1) * HD],
                                    start=(k == 0), stop=(k == DO - 1))
                        nc.vector.tensor_tensor(
                            out=O_sb[:, half * 512:(half + 1) * 512],
                            in0=po[:BL],
                            in1=bv_rep[:BL, half * 512:(half + 1) * 512], op=ADD)
                    OT = sb_small.tile([P, DO, BL], F32R, tag="OT")
                    transpose_into(lambda kb: O_sb[:, kb * P:(kb + 1) * P], OT,
                                   0, BL, DO)
                    pooled_sb = sb_keep.tile([BL, D], F32, tag="pooled_" + pref)
                    for n in range(2):
                        wu = stream_unit(pw["WoT"], 0, n * 512)
                        pm = ps_mm.tile([P, 512], F32, tag="mm_ps", name="pm")
                        for k in range(DO):
                            nc.tensor.matmul(pm[:BL], lhsT=OT[:, k, :],
                                             rhs=wu[:, k, :],
                                             start=(k == 0), stop=(k == DO - 1))
                        nc.vector.tensor_tensor(
                            out=pooled_sb[:, n * 512:(n + 1) * 512], in0=pm[:BL],
                            in1=ob_rep[:BL, n * 512:(n + 1) * 512], op=ADD)
                    pooled[pref] = pooled_sb

            # ======== assemble encoder input ========
            stage = dram.tile([NT, D], F32)
            st_v = stage[:].rearrange("(b t) d -> b t d", t=T)
            nc.sync.dma_start(st_v[:, 0, :], te)
            nc.sync.dma_start(st_v[:, 1, :], pooled["vl"])
            nc.sync.dma_start(st_v[:, 2, :], pooled["r"])
            nc.sync.dma_start(st_v[:, 3:, :], cond_e)
            x_enc = sb_keep.tile([NT, D], F32, tag="x_enc")
            nc.sync.dma_start(x_enc, stage[:])

            mask_sb = const.tile([NT, NT], F32)
            nc.sync.dma_start(mask_sb, enc_mask[:, :])

            layer_norm(NT, x_enc[:], D, eps5)

            # ======== encoder layers + head ========
            with tc.tile_pool(name="en_feat", bufs=1) as en_feat, \
                 tc.tile_pool(name="en_xt", bufs=1) as en_xt:

                def transpose_act(src, width, tag):
                    dst = en_xt.tile([P, width // P, NT], F32R, tag=tag,
                                     name=tag)
                    transpose_into(lambda kb: src[:, kb * P:(kb + 1) * P], dst,
                                   0, NT, width // P)
                    return dst

                def linear_nt(xT_t, wdram, nslices, brep, out_tile, act=None):
                    for n in range(nslices):
                        pm = ps_mm.tile([P, 512], F32, tag="mm_ps", name="pm")
                        wu = stream_unit(wdram, 0, n * 512)
                        for k in range(DO):
                            nc.tensor.matmul(
                                pm[:NT], lhsT=xT_t[:, k, :], rhs=wu[:, k, :],
                                start=(k == 0), stop=(k == DO - 1))
                        osl = out_tile[:, n * 512:(n + 1) * 512]
                        bsl = brep[:NT, n * 512:(n + 1) * 512]
                        if act is None:
                            nc.vector.tensor_tensor(out=osl, in0=pm[:NT], in1=bsl,
                                                    op=ADD)
                        else:
                            nc.vector.tensor_tensor(out=pm[:NT], in0=pm[:NT],
                                                    in1=bsl, op=ADD)
                            nc.scalar.activation(osl, pm[:NT], act)

                for li, L in enumerate(layers):
                    inb_rep = rep_vec(L["inb"], 3 * D, "inb", pool=en_feat)
                    outb_rep = rep_vec(L["outb"], D, "outb", pool=en_feat)
                    b2_rep = rep_vec(L["b2"], D, "b2", pool=en_feat)

                    h1 = en_feat.tile([NT, D], F32, tag="h1")
                    nc.vector.tensor_copy(out=h1, in_=x_enc)
                    layer_norm(NT, h1[:], D, eps5)
                    h1T = transpose_act(h1, D, "h1T")
                    qkv = en_feat.tile([NT, 3 * D], F32, tag="qkv")
                    for n in range(6):
                        pm = ps_mm.tile([P, 512], F32, tag="mm_ps", name="pm")
                        wu = stream_unit(L["WinT"], 0, n * 512)
                        for k in range(DO):
                            nc.tensor.matmul(pm[:NT], lhsT=h1T[:, k, :],
                                             rhs=wu[:, k, :],
                                             start=(k == 0), stop=(k == DO - 1))
                        nc.vector.tensor_tensor(
                            out=qkv[:, n * 512:(n + 1) * 512], in0=pm[:NT],
                            in1=inb_rep[:NT, n * 512:(n + 1) * 512], op=ADD)

                    qkT = en_xt.tile([P, 2 * DO, NT], F32, tag="qkT")
                    for kb in range(2 * DO):
                        sc = 0.125 if kb < DO else None
                        transpose_into(
                            lambda _, kb=kb: qkv[:, kb * P:(kb + 1) * P], qkT,
                            0, NT, 1, dcol=kb, scale=sc)
                    S_sb = en_feat.tile([NT, H, NT], F32, tag="S_sb")
                    for h in range(H):
                        base = HD * (h % 2)
                        psc = ps_s.tile([P, 512], F32, tag="s_ps", name="psc")
                        nc.tensor.matmul(psc[:NT, :NT],
                                         lhsT=qkT[base:base + HD, h // 2, :],
                                         rhs=qkT[base:base + HD, DO + h // 2, :],
                                         start=True, stop=True)
                        nc.vector.tensor_tensor(out=S_sb[:, h, :],
                                                in0=psc[:NT, :NT], in1=mask_sb,
                                                op=ADD)
                    nmax = sb_small.tile([NT, H], F32, tag="nmax_e")
                    nc.vector.tensor_reduce(out=nmax, in_=S_sb, axis=AX, op=MAX,
                                            negate=True)
                    nc.vector.tensor_tensor(
                        out=S_sb, in0=S_sb,
                        in1=nmax[:, :, None].to_broadcast((NT, H, NT)), op=ADD)
                    sflat = S_sb[:].rearrange("p a b -> p (a b)")
                    nc.scalar.activation(sflat, sflat, AF.Exp)
                    den = sb_small.tile([NT, H], F32, tag="den_e")
                    nc.vector.tensor_reduce(out=den, in_=S_sb, axis=AX, op=ADD)
                    nc.vector.reciprocal(out=den, in_=den)
                    nc.vector.tensor_tensor(
                        out=S_sb, in0=S_sb,
                        in1=den[:, :, None].to_broadcast((NT, H, NT)), op=MUL)

                    pav0 = ps_s.tile([P, 512], F32, tag="s_ps", name="pav0")
                    pav1 = ps_s.tile([P, 512], F32, tag="s_ps", name="pav1")
                    pavs = (pav0, pav1)
                    for h in range(H):
                        pt2 = t_psum()
                        nc.tensor.transpose(pt2[:NT, :NT], S_sb[:, h, :],
                                            ident[:NT, :NT])
                        aT = sb_scr.tile([NT, NT], F32, tag="aT", name="aT")
                        nc.vector.tensor_copy(out=aT, in_=pt2[:NT, :NT])
                        nc.tensor.matmul(
                            pavs[h // DO][:NT, (h % DO) * HD:(h % DO + 1) * HD],
                            lhsT=aT,
                            rhs=qkv[:, 2 * D + h * HD:2 * D + (h + 1) * HD],
                            start=True, stop=True)
                    ao = en_feat.tile([NT, D], F32, tag="ao")
                    for i in range(2):
                        nc.vector.tensor_copy(out=ao[:, i * 512:(i + 1) * 512],
                                              in_=pavs[i][:NT])
                    aoT = transpose_act(ao, D, "aoT")
                    attn_proj = en_feat.tile([NT, D], F32, tag="ao2")
                    linear_nt(aoT, L["WoutT"], 2, outb_rep, attn_proj)
                    nc.vector.tensor_tensor(out=x_enc, in0=x_enc, in1=attn_proj,
                                            op=ADD)

                    # FFN in two 2048-halves to bound SBUF
                    h2 = en_feat.tile([NT, D], F32, tag="h1")
                    nc.vector.tensor_copy(out=h2, in_=x_enc)
                    layer_norm(NT, h2[:], D, eps5)
                    h2T = transpose_act(h2, D, "h1T")
                    pm0 = ps_mm.tile([P, 512], F32, tag="mm_ps", name="pm0")
                    pm1 = ps_mm.tile([P, 512], F32, tag="mm_ps", name="pm1")
                    pmn = (pm0, pm1)
                    for half in range(2):
                        b1h = rep_vec(L["b1"], 2048, "b1h", pool=en_feat,
                                      col0=half * 2048)
                        g_half = en_feat.tile([NT, 2048], F32, tag="g_half")
                        for u in range(4):
                            pmg = ps_s.tile([P, 512], F32, tag="s_ps", name="pmg")
                            wu = stream_unit(L["W1T"], 0, half * 2048 + u * 512)
                            for k in range(DO):
                                nc.tensor.matmul(pmg[:NT], lhsT=h2T[:, k, :],
                                                 rhs=wu[:, k, :],
                                                 start=(k == 0), stop=(k == DO - 1))
                            nc.vector.tensor_tensor(
                                out=pmg[:NT], in0=pmg[:NT],
                                in1=b1h[:NT, u * 512:(u + 1) * 512], op=ADD)
                            nc.scalar.activation(
                                g_half[:, u * 512:(u + 1) * 512], pmg[:NT],
                                AF.Gelu)
                        gTh = en_xt.tile([P, 16, NT], F32R, tag="gTh",
                                         name="gTh")
                        transpose_into(
                            lambda kb: g_half[:, kb * P:(kb + 1) * P], gTh, 0,
                            NT, 16)
                        for kg in range(2):
                            for n in range(2):
                                wu2 = stream_unit(L["W2T"],
                                                  (half * 2 + kg) * D, n * 512)
                                for k in range(DO):
                                    nc.tensor.matmul(
                                        pmn[n][:NT],
                                        lhsT=gTh[:, kg * DO + k, :],
                                        rhs=wu2[:, k, :],
                                        start=(half == 0 and kg == 0 and k == 0),
                                        stop=(half == 1 and kg == 1 and
                                              k == DO - 1))
                    for n in range(2):
                        nc.vector.tensor_tensor(
                            out=pmn[n][:NT], in0=pmn[n][:NT],
                            in1=b2_rep[:NT, n * 512:(n + 1) * 512], op=ADD)
                        nc.vector.tensor_tensor(
                            out=x_enc[:, n * 512:(n + 1) * 512],
                            in0=x_enc[:, n * 512:(n + 1) * 512],
                            in1=pmn[n][:NT], op=ADD)

                # ---- head ----
                mp1b_rep = rep_vec(mp1b, D, "outb", pool=en_feat)
                mp2b_rep = rep_vec(mp2b, D, "b2", pool=en_feat)
                layer_norm(NT, x_enc[:], D, eps5)
                xT2 = transpose_act(x_enc, D, "h1T")
                hmid = en_feat.tile([NT, D], F32, tag="h1")
                linear_nt(xT2, mp1T, 2, mp1b_rep, hmid, act=AF.Gelu)
                hT = transpose_act(hmid, D, "aoT")
                y_sb = en_feat.tile([NT, D], F32, tag="ao2")
                linear_nt(hT, mp2T, 2, mp2b_rep, y_sb)
                nc.sync.dma_start(y_out[:, :], y_sb)

    nc.compile()
    return nc


# ---------------------------------------------------------------- host glue
def _prep_arrays(params):
    p = params
    a = {}

    def npf(x):
        return np.ascontiguousarray(np.asarray(x, dtype=np.float32))

    a["Wt_vl"] = npf(p["vl_proj_w"]).T.copy()
    a["pb_vl"] = npf(p["vl_proj_b"]).reshape(1, D)
    a["Wt_r"] = npf(p["r_proj_w"]).T.copy()
    a["pb_r"] = npf(p["r_proj_b"]).reshape(1, D)
    a["WtC"] = npf(p["cond_obs_w"]).T.copy()
    a["pbC"] = npf(p["cond_obs_b"]).reshape(1, D)

    for pref in ("vl", "r"):
        in_w = npf(p[pref + "_in_w"])
        in_b = npf(p[pref + "_in_b"])
        wq, wk, wv = in_w[:D], in_w[D:2 * D], in_w[2 * D:]
        bq, bv = in_b[:D], in_b[2 * D:]
        q = npf(p[pref + "_pool_query"]).reshape(D)
        qn_w = npf(p[pref + "_qn_w"])
        kn_w = npf(p[pref + "_kn_w"])
        qh = q.reshape(H, HD)
        qn = (qh / np.sqrt((qh ** 2).mean(-1, keepdims=True) + 1e-6) * qn_w
              ).reshape(D)
        qproj = wq @ qn + bq
        U = np.zeros((H, D), np.float32)
        knw_t = np.tile(kn_w, H)
        for h in range(H):
            u0 = wk[h * HD:(h + 1) * HD].T @ qproj[h * HD:(h + 1) * HD]
            U[h] = u0 * knw_t / np.sqrt(HD)
        a["UT_" + pref] = U.T.copy()
        a["wvT_" + pref] = wv.T.copy()
        a["bv_" + pref] = bv.reshape(1, D)
        a["WoT_" + pref] = npf(p[pref + "_out_w"]).T.copy()
        a["ob_" + pref] = npf(p[pref + "_out_b"]).reshape(1, D)

    for i, L in enumerate(p["layers"]):
        a[f"L{i}_WinT"] = npf(L["in_w"]).T.copy()
        a[f"L{i}_inb"] = npf(L["in_b"]).reshape(1, 3 * D)
        a[f"L{i}_WoutT"] = npf(L["out_w"]).T.copy()
        a[f"L{i}_outb"] = npf(L["out_b"]).reshape(1, D)
        a[f"L{i}_W1T"] = npf(L["lin1_w"]).T.copy()
        a[f"L{i}_b1"] = npf(L["lin1_b"]).reshape(1, DFF)
        a[f"L{i}_W2T"] = npf(L["lin2_w"]).T.copy()
        a[f"L{i}_b2"] = npf(L["lin2_b"]).reshape(1, D)
        for nm in ("ln1_w", "ln2_w"):
            assert np.all(np.asarray(L[nm]) == 1.0), "non-trivial ln weight"
        for nm in ("ln1_b", "ln2_b"):
            assert np.all(np.asarray(L[nm]) == 0.0), "non-trivial ln bias"

    for nm in ("vl_ln_w", "r_ln_w", "pre_ln_w", "mem_ln_w"):
        assert np.all(np.asarray(p[nm]) == 1.0), "non-trivial ln weight"
    for nm in ("vl_ln_b", "r_ln_b", "pre_ln_b", "mem_ln_b"):
        assert np.all(np.asarray(p[nm]) == 0.0), "non-trivial ln bias"
    assert np.all(np.asarray(p["cond_pos_emb"]) == 0.0), "non-trivial pos emb"

    a["mp1T"] = npf(p["mp1_w"]).T.copy()
    a["mp1b"] = npf(p["mp1_b"]).reshape(1, D)
    a["mp2T"] = npf(p["mp2_w"]).T.copy()
    a["mp2b"] = npf(p["mp2_b"]).reshape(1, D)

    half = D // 2
    scale = np.log(10000.0) / (half - 1)
    a["freqs"] = np.exp(np.arange(half, dtype=np.float32) * -scale
                        ).reshape(1, half).astype(np.float32)

    mask = np.full((NT, NT), -1e9, np.float32)
    for b in range(BL):
        mask[b * T:(b + 1) * T, b * T:(b + 1) * T] = 0.0
    a["enc_mask"] = mask
    return a


def kernel(timesteps, cond, vl_features, reasoning_features, params):
    timesteps = np.asarray(timesteps, np.float32)
    cond = np.asarray(cond, np.float32)
    vl = np.asarray(vl_features, np.float32)
    rs = np.asarray(reasoning_features, np.float32)

    if "nc" not in _CACHE:
        _CACHE["nc"] = build_nc()
    nc = _CACHE["nc"]

    shared = _prep_arrays(params)
    in_maps = []
    for c in range(NCORES):
        bs = slice(c * BL, (c + 1) * BL)
        m = dict(shared)
        m["x_vl"] = np.ascontiguousarray(vl[bs].reshape(BL * S, KVL))
        m["x_r"] = np.ascontiguousarray(rs[bs].reshape(BL * S, KVL))
        m["x_cond"] = np.ascontiguousarray(cond[bs].reshape(BL * TO, 512))
        m["tvec"] = np.ascontiguousarray(timesteps[bs].reshape(BL, 1))
        in_maps.append(m)

    res = run_bass_kernel_spmd(nc, in_maps, core_ids=list(range(NCORES)))
    _CACHE["last_res"] = res
    out = np.concatenate([r["y"].reshape(BL, T, D) for r in res.results], axis=0)
    return out


# revision 12
# speedup vs baseline: 36.7367x; 36.7367x over previous
"""Trainium2 Bass kernel for nn_CustomEncoderBlock (dense transformer encoder).

Sharding: pure data parallel over batch (64) across 8 NeuronCores, params
replicated. Token-major activations; PE-transposes feed f32r matmuls.
Pool attention is algebraically folded: since q_len==1 and the pool query is
input-independent, the packed k-projection collapses into U @ rms(feat) and
the v-projection is applied after pooling (per-head weighted feature sums).
Encoder attention runs over all 88 packed tokens with a block-diagonal mask.
"""
import numpy as np

import concourse.bass as bass
import concourse.tile as tile
import concourse.mybir as mybir
from concourse import bacc
from concourse.bass_utils import run_bass_kernel_spmd
from concourse.masks import make_identity

F32 = mybir.dt.float32
F32R = mybir.dt.float32r
I32 = mybir.dt.int32
ADD = mybir.AluOpType.add
SUB = mybir.AluOpType.subtract
MUL = mybir.AluOpType.mult
MAX = mybir.AluOpType.max
AX = mybir.AxisListType.X
AF = mybir.ActivationFunctionType

NCORES = 8
B, TO, S, KVL = 64, 8, 512, 1536
D, H, HD = 1024, 16, 64
BL = B // NCORES
T = 3 + TO
NT = BL * T                      # 88 tokens per core
DFF = 4 * D
NL = 4
P = 128
KO = KVL // P                    # 12
DO = D // P                      # 8
TWO_PI = float(2 * np.pi)
HALF_PI = float(np.pi / 2)

_CACHE = {}


def build_nc():
    nc = bacc.Bacc(None, target_bir_lowering=False)

    def din(name, shape, dtype=F32):
        return nc.dram_tensor(name, list(shape), dtype, kind="ExternalInput")

    x_vl = din("x_vl", (BL * S, KVL))
    x_r = din("x_r", (BL * S, KVL))
    x_cond = din("x_cond", (BL * TO, 512))
    tvec = din("tvec", (BL, 1))
    freqs = din("freqs", (1, 512))
    enc_mask = din("enc_mask", (NT, NT))

    per_pref = {}
    for pref in ("vl", "r"):
        per_pref[pref] = dict(
            Wt=din(f"Wt_{pref}", (KVL, D), F32R),
            pb=din(f"pb_{pref}", (1, D)),
            UT=din(f"UT_{pref}", (D, H), F32R),
            wvT=din(f"wvT_{pref}", (D, D), F32R),
            bv=din(f"bv_{pref}", (1, D)),
            WoT=din(f"WoT_{pref}", (D, D), F32R),
            ob=din(f"ob_{pref}", (1, D)),
        )
    WtC = din("WtC", (512, D), F32R)
    pbC = din("pbC", (1, D))

    layers = []
    for i in range(NL):
        layers.append(dict(
            WinT=din(f"L{i}_WinT", (D, 3 * D), F32R),
            inb=din(f"L{i}_inb", (1, 3 * D)),
            WoutT=din(f"L{i}_WoutT", (D, D), F32R),
            outb=din(f"L{i}_outb", (1, D)),
            W1T=din(f"L{i}_W1T", (D, DFF), F32R),
            b1=din(f"L{i}_b1", (1, DFF)),
            W2T=din(f"L{i}_W2T", (DFF, D), F32R),
            b2=din(f"L{i}_b2", (1, D)),
        ))
    mp1T = din("mp1T", (D, D), F32R)
    mp1b = din("mp1b", (1, D))
    mp2T = din("mp2T", (D, D), F32R)
    mp2b = din("mp2b", (1, D))

    y_out = nc.dram_tensor("y", [NT, D], F32, kind="ExternalOutput")

    with tile.TileContext(nc) as tc:
        with tc.tile_pool(name="const", bufs=1) as const, \
             tc.tile_pool(name="sb_keep", bufs=1) as sb_keep, \
             tc.tile_pool(name="sb_small", bufs=1) as sb_small, \
             tc.tile_pool(name="sb_scr", bufs=1) as sb_scr, \
             tc.tile_pool(name="sb_stream", bufs=2) as sb_stream, \
             tc.tile_pool(name="ps_mm", bufs=2, space="PSUM") as ps_mm, \
             tc.tile_pool(name="ps_t", bufs=4, space="PSUM") as ps_t, \
             tc.tile_pool(name="ps_s", bufs=2, space="PSUM") as ps_s, \
             tc.tile_pool(name="dram", bufs=1, space="DRAM") as dram:

            ident = const.tile([P, P], F32)
            make_identity(nc, ident)
            eps5 = const.tile([P, 1], F32)
            nc.vector.memset(eps5, 1e-5)
            eps6 = const.tile([P, 1], F32)
            nc.vector.memset(eps6, 1e-6)

            def t_psum():
                return ps_t.tile([P, 512], F32, tag="t_ps", name="t_ps")

            def rep_vec(dram_vec, n, tag, pool=None, col0=0):
                """[1,n] slice of a DRAM vector -> [128,n] broadcast f32 tile."""
                rep = (pool or sb_keep).tile([P, n], F32, tag="rep_" + tag,
                                             name="rep_" + tag)
                src = dram_vec[:, col0:col0 + n]
                bc = bass.AP(tensor=src.tensor, offset=src.offset,
                             ap=[[0, P], [1, n]])
                nc.gpsimd.dma_start(out=rep, in_=bc)
                return rep

            def transpose_into(src_fn, dst, dst_j, rows, kblocks, dcol=None,
                               scale=None):
                for kb in range(kblocks):
                    pt = t_psum()
                    nc.tensor.transpose(pt[:, :rows], src_fn(kb),
                                        ident[:rows, :rows])
                    col = kb if dcol is None else dcol
                    dsl = dst[:, col, dst_j * rows:(dst_j + 1) * rows]
                    if scale is None:
                        # alternate eviction engine to decouple PE from DVE
                        if kb % 2 == 0:
                            nc.vector.tensor_copy(out=dsl, in_=pt[:, :rows])
                        else:
                            nc.scalar.activation(dsl, pt[:, :rows], AF.Copy)
                    else:
                        nc.vector.tensor_scalar(out=dsl, in0=pt[:, :rows],
                                                scalar1=scale, scalar2=None,
                                                op0=MUL)

            def layer_norm(rows, x_ap, width, eps_tile):
                ssum = sb_small.tile([P, 1], F32, tag="ln_sum", name="ln_sum")
                nc.vector.tensor_reduce(out=ssum[:rows], in_=x_ap, axis=AX, op=ADD)
                sq = sb_scr.tile([P, 1024], F32, tag="sq1024", name="sq")
                ssq = sb_small.tile([P, 1], F32, tag="ln_ssq", name="ln_ssq")
                nc.scalar.activation(sq[:rows, :width], x_ap, AF.Square,
                                     accum_out=ssq[:rows])
                nmean = sb_small.tile([P, 1], F32, tag="ln_nm", name="ln_nm")
                nc.vector.tensor_scalar(out=nmean[:rows], in0=ssum[:rows],
                                        scalar1=-1.0 / width, scalar2=None, op0=MUL)
                m2 = sb_small.tile([P, 1], F32, tag="ln_m2", name="ln_m2")
                nc.vector.tensor_tensor(out=m2[:rows], in0=nmean[:rows],
                                        in1=nmean[:rows], op=MUL)
                var = sb_small.tile([P, 1], F32, tag="ln_var", name="ln_var")
                nc.vector.tensor_scalar(out=var[:rows], in0=ssq[:rows],
                                        scalar1=1.0 / width, scalar2=m2[:rows],
                                        op0=MUL, op1=SUB)
                nc.scalar.activation(var[:rows], var[:rows], AF.Sqrt,
                                     bias=eps_tile[:rows])
                nc.vector.reciprocal(out=var[:rows], in_=var[:rows])
                nc.vector.tensor_scalar(out=x_ap, in0=x_ap, scalar1=nmean[:rows],
                                        scalar2=var[:rows], op0=ADD, op1=MUL)

            def stream_unit(wdram, k0, n0, tag="wunit"):
                u = sb_stream.tile([P, DO, 512], F32R, tag=tag, name=tag)
                nc.sync.dma_start(
                    u, wdram[k0:k0 + D, n0:n0 + 512]
                    .rearrange("(c p) n -> p c n", p=P))
                return u

            # ======== t_emb ========
            te = sb_keep.tile([BL, D], F32, tag="te")
            with tc.tile_pool(name="temb", bufs=1) as temb:
                fr_rep = temb.tile([BL, 512], F32, tag="fr_rep")
                fsrc = freqs[:, :]
                nc.gpsimd.dma_start(out=fr_rep, in_=bass.AP(
                    tensor=fsrc.tensor, offset=fsrc.offset, ap=[[0, BL], [1, 512]]))
                t_sb = temb.tile([BL, 1], F32, tag="tvec")
                nc.sync.dma_start(t_sb, tvec[:, :])
                ang = temb.tile([BL, 512], F32, tag="ang")
                nc.vector.tensor_scalar_mul(ang, fr_rep, t_sb)
                for half, shift in ((0, 0.0), (1, HALF_PI)):
                    a2 = temb.tile([BL, 512], F32, tag="a2")
                    nc.vector.tensor_scalar(out=a2, in0=ang, scalar1=shift,
                                            scalar2=1.0 / TWO_PI, op0=ADD, op1=MUL)
                    mi = temb.tile([BL, 512], I32, tag="mi")
                    nc.vector.tensor_copy(out=mi, in_=a2)
                    mf = temb.tile([BL, 512], F32, tag="mf")
                    nc.vector.tensor_copy(out=mf, in_=mi)
                    nc.vector.tensor_tensor(out=mf, in0=a2, in1=mf, op=SUB)
                    nc.vector.tensor_scalar(out=mf, in0=mf, scalar1=TWO_PI,
                                            scalar2=None, op0=MUL)
                    nc.scalar.activation(te[:, half * 512:(half + 1) * 512], mf,
                                         AF.Sin)

            # ======== stage A ========
            pooled = {}
            with tc.tile_pool(name="sa_w", bufs=1) as sa_w, \
                 tc.tile_pool(name="sa_keep", bufs=1) as sa_keep, \
                 tc.tile_pool(name="sa_feat", bufs=1) as sa_feat, \
                 tc.tile_pool(name="sa_x", bufs=2) as sa_x:

                # ---- cond projection ----
                cond_e = sb_keep.tile([BL * TO, D], F32, tag="cond_e")
                with tc.tile_pool(name="sa_cond", bufs=1) as sa_cond:
                    WtC_sb = sa_cond.tile([P, 4, D], F32R, tag="WtC")
                    nc.sync.dma_start(WtC_sb,
                                      WtC[:, :].rearrange("(c p) n -> p c n", p=P))
                    pbC_rep = rep_vec(pbC, D, "pbC", pool=sa_cond)
                    xc = sa_x.tile([BL * TO, 512], F32, tag="xc")
                    nc.sync.dma_start(xc, x_cond[:, :])
                    condT = sa_cond.tile([P, 4, BL * TO], F32R, tag="condT")
                    transpose_into(lambda kb: xc[:, kb * P:(kb + 1) * P], condT,
                                   0, BL * TO, 4)
                    for n in range(2):
                        pm = ps_mm.tile([P, 512], F32, tag="mm_ps", name="pm")
                        for j in range(4):
                            nc.tensor.matmul(
                                pm[:BL * TO], lhsT=condT[:, j, :],
                                rhs=WtC_sb[:, j, n * 512:(n + 1) * 512],
                                start=(j == 0), stop=(j == 3))
                        nc.vector.tensor_tensor(
                            out=cond_e[:, n * 512:(n + 1) * 512],
                            in0=pm[:BL * TO],
                            in1=pbC_rep[:BL * TO, n * 512:(n + 1) * 512], op=ADD)

                # ---- per-modality projection + pool ----
                for pref, xin in (("vl", x_vl), ("r", x_r)):
                    pw = per_pref[pref]
                    W_sb = sa_w.tile([P, KO, D], F32R, tag="Wbig")
                    nc.sync.dma_start(
                        W_sb, pw["Wt"][:, :].rearrange("(c p) n -> p c n", p=P))
                    pb_rep = rep_vec(pw["pb"], D, "pb", pool=sa_keep)
                    UT_sb = sa_w.tile([P, DO, H], F32R, tag="UT")
                    nc.sync.dma_start(
                        UT_sb, pw["UT"][:, :].rearrange("(c p) n -> p c n", p=P))
                    bv_rep = rep_vec(pw["bv"], D, "bv", pool=sa_keep)
                    ob_rep = rep_vec(pw["ob"], D, "ob", pool=sa_keep)

                    GT = sa_keep.tile([P, DO, H, BL], F32R, tag="GT")
                    xin_v = xin[:, :].rearrange("(b s) k -> b s k", s=S)

                    for b in range(BL):
                        feat = sa_feat.tile([P, S // P, D], F32R, tag="feat")
                        for i in range(S // P):
                            x_in = sa_x.tile([P, KVL], F32, tag="x_in")
                            nc.sync.dma_start(x_in, xin_v[b, i * P:(i + 1) * P, :])
                            xT_c = sa_x.tile([P, KO, P], F32R, tag="xT_c")
                            transpose_into(
                                lambda kb: x_in[:, kb * P:(kb + 1) * P],
                                xT_c, 0, P, KO)
                            for n in range(2):
                                pm = ps_mm.tile([P, 512], F32, tag="mm_ps",
                                                name="pm")
                                for j in range(KO):
                                    nc.tensor.matmul(
                                        pm, lhsT=xT_c[:, j, :],
                                        rhs=W_sb[:, j, n * 512:(n + 1) * 512],
                                        start=(j == 0), stop=(j == KO - 1))
                                nc.vector.tensor_tensor(
                                    out=feat[:, i, n * 512:(n + 1) * 512],
                                    in0=pm, in1=pb_rep[:, n * 512:(n + 1) * 512],
                                    op=ADD)
                            layer_norm(P, feat[:, i, :], D, eps5)

                        # per-head rms rstd [128, 4, 16]
                        rstd = sb_small.tile([P, S // P, H], F32, tag="rstd")
                        for i in range(S // P):
                            sq = sb_scr.tile([P, 1024], F32, tag="sq1024",
                                             name="sq")
                            nc.scalar.activation(sq, feat[:, i, :], AF.Square)
                            nc.vector.tensor_reduce(
                                out=rstd[:, i, :],
                                in_=sq[:].rearrange("p (h d) -> p h d", d=HD),
                                axis=AX, op=ADD)
                        rsf = rstd[:].rearrange("p a b -> p (a b)")
                        nc.scalar.activation(rsf, rsf, AF.Sqrt, bias=eps6,
                                             scale=1.0 / HD)
                        nc.vector.reciprocal(out=rsf, in_=rsf)

                        # kn per s-chunk -> knT_i [128, 8, 128] f32r -> scores
                        psc = ps_s.tile([P, 512], F32, tag="s_ps", name="psc")
                        for i in range(S // P):
                            knT_i = sa_x.tile([P, DO, P], F32R, tag="knT_i")
                            for m in range(DO):
                                knb = sb_scr.tile([P, P], F32, tag="knb",
                                                  name="knb")
                                nc.vector.tensor_tensor(
                                    out=knb[:].rearrange("p (a b) -> p a b", b=HD),
                                    in0=feat[:, i, m * P:(m + 1) * P].rearrange(
                                        "p (a b) -> p a b", b=HD),
                                    in1=rstd[:, i, 2 * m:2 * m + 2, None]
                                    .to_broadcast((P, 2, HD)),
                                    op=MUL)
                                pt2 = t_psum()
                                nc.tensor.transpose(pt2[:, :P], knb, ident)
                                nc.vector.tensor_copy(out=knT_i[:, m, :],
                                                      in_=pt2[:, :P])
                            for m in range(DO):
                                nc.tensor.matmul(
                                    psc[:H, i * P:(i + 1) * P],
                                    lhsT=UT_sb[:, m, :], rhs=knT_i[:, m, :],
                                    start=(m == 0), stop=(m == DO - 1))
                        nmax = sb_small.tile([H, 1], F32, tag="nmax")
                        nc.vector.tensor_reduce(out=nmax, in_=psc[:H], axis=AX,
                                                op=MAX, negate=True)
                        attn = sb_small.tile([H, S], F32, tag="attn")
                        den = sb_small.tile([H, 1], F32, tag="den")
                        nc.scalar.activation(attn, psc[:H], AF.Exp, bias=nmax,
                                             accum_out=den)
                        nc.vector.reciprocal(out=den, in_=den)
                        nc.vector.tensor_scalar_mul(attn, attn, den)
                        attnT = sb_small.tile([P, S // P, H], F32R, tag="attnT")
                        transpose_into(lambda kb: attn[:, kb * P:(kb + 1) * P],
                                       attnT, 0, H, S // P)
                        for m in range(DO):
                            pg = ps_s.tile([P, 512], F32, tag="s_ps", name="pg")
                            for i in range(S // P):
                                nc.tensor.matmul(
                                    pg[:, :H],
                                    lhsT=feat[:, i, m * P:(m + 1) * P],
                                    rhs=attnT[:, i, :],
                                    start=(i == 0), stop=(i == S // P - 1))
                            nc.vector.tensor_copy(out=GT[:, m, :, b],
                                                  in_=pg[:, :H])

                    # ---- apply wv per head, then out-proj ----
                    O_sb = sb_small.tile([BL, D], F32, tag="O_sb")
                    for half in range(2):
                        po = ps_mm.tile([P, 512], F32, tag="mm_ps", name="po")
                        wu = stream_unit(pw["wvT"], 0, half * 512)
                        for h8 in range(DO):
                            h = half * DO + h8
                            for k in range(DO):
                                nc.tensor.matmul(
                                    po[:BL, h8 * HD:(h8 + 1) * HD],
                                    lhsT=GT[:, k, h, :],
                                    rhs=wu[:, k, h8 * HD:(h8 + 1) * HD],
                                    start=(k == 0), stop=(k == DO - 1))
                        nc.vector.tensor_tensor(
                            out=O_sb[:, half * 512:(half + 1) * 512],
                            in0=po[:BL],
                            in1=bv_rep[:BL, half * 512:(half + 1) * 512], op=ADD)
                    OT = sb_small.tile([P, DO, BL], F32R, tag="OT")
                    transpose_into(lambda kb: O_sb[:, kb * P:(kb + 1) * P], OT,
                                   0, BL, DO)
                    pooled_sb = sb_keep.tile([BL, D], F32, tag="pooled_" + pref)
                    for n in range(2):
                        wu = stream_unit(pw["WoT"], 0, n * 512)
                        pm = ps_mm.tile([P, 512], F32, tag="mm_ps", name="pm")
                        for k in range(DO):
                            nc.tensor.matmul(pm[:BL], lhsT=OT[:, k, :],
                                             rhs=wu[:, k, :],
                                             start=(k == 0), stop=(k == DO - 1))
                        nc.vector.tensor_tensor(
                            out=pooled_sb[:, n * 512:(n + 1) * 512], in0=pm[:BL],
                            in1=ob_rep[:BL, n * 512:(n + 1) * 512], op=ADD)
                    pooled[pref] = pooled_sb

            # ======== assemble encoder input ========
            stage = dram.tile([NT, D], F32)
            st_v = stage[:].rearrange("(b t) d -> b t d", t=T)
            nc.sync.dma_start(st_v[:, 0, :], te)
            nc.sync.dma_start(st_v[:, 1, :], pooled["vl"])
            nc.sync.dma_start(st_v[:, 2, :], pooled["r"])
            nc.sync.dma_start(st_v[:, 3:, :], cond_e)
            x_enc = sb_keep.tile([NT, D], F32, tag="x_enc")
            nc.sync.dma_start(x_enc, stage[:])

            mask_sb = const.tile([NT, NT], F32)
            nc.sync.dma_start(mask_sb, enc_mask[:, :])

            layer_norm(NT, x_enc[:], D, eps5)

            # ======== encoder layers + head ========
            with tc.tile_pool(name="en_feat", bufs=1) as en_feat, \
                 tc.tile_pool(name="en_xt", bufs=1) as en_xt:

                def transpose_act(src, width, tag):
                    dst = en_xt.tile([P, width // P, NT], F32R, tag=tag,
                                     name=tag)
                    transpose_into(lambda kb: src[:, kb * P:(kb + 1) * P], dst,
                                   0, NT, width // P)
                    return dst

                def linear_nt(xT_t, wdram, nslices, brep, out_tile, act=None):
                    for n in range(nslices):
                        pm = ps_mm.tile([P, 512], F32, tag="mm_ps", name="pm")
                        wu = stream_unit(wdram, 0, n * 512)
                        for k in range(DO):
                            nc.tensor.matmul(
                                pm[:NT], lhsT=xT_t[:, k, :], rhs=wu[:, k, :],
                                start=(k == 0), stop=(k == DO - 1))
                        osl = out_tile[:, n * 512:(n + 1) * 512]
                        bsl = brep[:NT, n * 512:(n + 1) * 512]
                        if act is None:
                            nc.vector.tensor_tensor(out=osl, in0=pm[:NT], in1=bsl,
                                                    op=ADD)
                        else:
                            nc.vector.tensor_tensor(out=pm[:NT], in0=pm[:NT],
                                                    in1=bsl, op=ADD)
                            nc.scalar.activation(osl, pm[:NT], act)

                for li, L in enumerate(layers):
                    inb_rep = rep_vec(L["inb"], 3 * D, "inb", pool=en_feat)
                    outb_rep = rep_vec(L["outb"], D, "outb", pool=en_feat)
                    b2_rep = rep_vec(L["b2"], D, "b2", pool=en_feat)

                    h1 = en_feat.tile([NT, D], F32, tag="h1")
                    nc.vector.tensor_copy(out=h1, in_=x_enc)
                    layer_norm(NT, h1[:], D, eps5)
                    h1T = transpose_act(h1, D, "h1T")
                    qkv = en_feat.tile([NT, 3 * D], F32, tag="qkv")
                    for n in range(6):
                        pm = ps_mm.tile([P, 512], F32, tag="mm_ps", name="pm")
                        wu = stream_unit(L["WinT"], 0, n * 512)
                        for k in range(DO):
                            nc.tensor.matmul(pm[:NT], lhsT=h1T[:, k, :],
                                             rhs=wu[:, k, :],
                                             start=(k == 0), stop=(k == DO - 1))
                        nc.vector.tensor_tensor(
                            out=qkv[:, n * 512:(n + 1) * 512], in0=pm[:NT],
                            in1=inb_rep[:NT, n * 512:(n + 1) * 512], op=ADD)

                    qkT = en_xt.tile([P, 2 * DO, NT], F32, tag="qkT")
                    for kb in range(2 * DO):
                        sc = 0.125 if kb < DO else None
                        transpose_into(
                            lambda _, kb=kb: qkv[:, kb * P:(kb + 1) * P], qkT,
                            0, NT, 1, dcol=kb, scale=sc)
                    S_sb = en_feat.tile([NT, H, NT], F32, tag="S_sb")
                    for h in range(H):
                        base = HD * (h % 2)
                        psc = ps_s.tile([P, 512], F32, tag="s_ps", name="psc")
                        nc.tensor.matmul(psc[:NT, :NT],
                                         lhsT=qkT[base:base + HD, h // 2, :],
                                         rhs=qkT[base:base + HD, DO + h // 2, :],
                                         start=True, stop=True)
                        nc.vector.tensor_tensor(out=S_sb[:, h, :],
                                                in0=psc[:NT, :NT], in1=mask_sb,
                                                op=ADD)
                    nmax = sb_small.tile([NT, H], F32, tag="nmax_e")
                    nc.vector.tensor_reduce(out=nmax, in_=S_sb, axis=AX, op=MAX,
                                            negate=True)
                    nc.vector.tensor_tensor(
                        out=S_sb, in0=S_sb,
                        in1=nmax[:, :, None].to_broadcast((NT, H, NT)), op=ADD)
                    sflat = S_sb[:].rearrange("p a b -> p (a b)")
                    nc.scalar.activation(sflat, sflat, AF.Exp)
                    den = sb_small.tile([NT, H], F32, tag="den_e")
                    nc.vector.tensor_reduce(out=den, in_=S_sb, axis=AX, op=ADD)
                    nc.vector.reciprocal(out=den, in_=den)
                    nc.vector.tensor_tensor(
                        out=S_sb, in0=S_sb,
                        in1=den[:, :, None].to_broadcast((NT, H, NT)), op=MUL)

                    pav0 = ps_s.tile([P, 512], F32, tag="s_ps", name="pav0")
                    pav1 = ps_s.tile([P, 512], F32, tag="s_ps", name="pav1")
                    pavs = (pav0, pav1)
                    for h in range(H):
                        pt2 = t_psum()
                        nc.tensor.transpose(pt2[:NT, :NT], S_sb[:, h, :],
                                            ident[:NT, :NT])
                        aT = sb_scr.tile([NT, NT], F32, tag="aT", name="aT")
                        nc.vector.tensor_copy(out=aT, in_=pt2[:NT, :NT])
                        nc.tensor.matmul(
                            pavs[h // DO][:NT, (h % DO) * HD:(h % DO + 1) * HD],
                            lhsT=aT,
                            rhs=qkv[:, 2 * D + h * HD:2 * D + (h + 1) * HD],
                            start=True, stop=True)
                    ao = en_feat.tile([NT, D], F32, tag="ao")
                    for i in range(2):
                        nc.vector.tensor_copy(out=ao[:, i * 512:(i + 1) * 512],
                                              in_=pavs[i][:NT])
                    aoT = transpose_act(ao, D, "aoT")
                    attn_proj = en_feat.tile([NT, D], F32, tag="ao2")
                    linear_nt(aoT, L["WoutT"], 2, outb_rep, attn_proj)
                    nc.vector.tensor_tensor(out=x_enc, in0=x_enc, in1=attn_proj,
                                            op=ADD)

                    # FFN in two 2048-halves to bound SBUF
                    h2 = en_feat.tile([NT, D], F32, tag="h1")
                    nc.vector.tensor_copy(out=h2, in_=x_enc)
                    layer_norm(NT, h2[:], D, eps5)
                    h2T = transpose_act(h2, D, "h1T")
                    pm0 = ps_mm.tile([P, 512], F32, tag="mm_ps", name="pm0")
                    pm1 = ps_mm.tile([P, 512], F32, tag="mm_ps", name="pm1")
                    pmn = (pm0, pm1)
                    for half in range(2):
                        b1h = rep_vec(L["b1"], 2048, "b1h", pool=en_feat,
                                      col0=half * 2048)
                        g_half = en_feat.tile([NT, 2048], F32, tag="g_half")
                        for u in range(4):
                            pmg = ps_s.tile([P, 512], F32, tag="s_ps", name="pmg")
                            wu = stream_unit(L["W1T"], 0, half * 2048 + u * 512)
                            for k in range(DO):
                                nc.tensor.matmul(pmg[:NT], lhsT=h2T[:, k, :],
                                                 rhs=wu[:, k, :],
                                                 start=(k == 0), stop=(k == DO - 1))
                            nc.vector.tensor_tensor(
                                out=pmg[:NT], in0=pmg[:NT],
                                in1=b1h[:NT, u * 512:(u + 1) * 512], op=ADD)
                            nc.scalar.activation(
                                g_half[:, u * 512:(u + 1) * 512], pmg[:NT],
                                AF.Gelu)
                        gTh = en_xt.tile([P, 16, NT], F32R, tag="gTh",
                                         name="gTh")
                        transpose_into(
                            lambda kb: g_half[:, kb * P:(kb + 1) * P], gTh, 0,
                            NT, 16)
                        for kg in range(2):
                            for n in range(2):
                                wu2 = stream_unit(L["W2T"],
                                                  (half * 2 + kg) * D, n * 512)
                                for k in range(DO):
                                    nc.tensor.matmul(
                                        pmn[n][:NT],
                                        lhsT=gTh[:, kg * DO + k, :],
                                        rhs=wu2[:, k, :],
                                        start=(half == 0 and kg == 0 and k == 0),
                                        stop=(half == 1 and kg == 1 and
                                              k == DO - 1))
                    for n in range(2):
                        nc.vector.tensor_tensor(
                            out=pmn[n][:NT], in0=pmn[n][:NT],
                            in1=b2_rep[:NT, n * 512:(n + 1) * 512], op=ADD)
                        nc.vector.tensor_tensor(
                            out=x_enc[:, n * 512:(n + 1) * 512],
                            in0=x_enc[:, n * 512:(n + 1) * 512],
                            in1=pmn[n][:NT], op=ADD)

                # ---- head ----
                mp1b_rep = rep_vec(mp1b, D, "outb", pool=en_feat)
                mp2b_rep = rep_vec(mp2b, D, "b2", pool=en_feat)
                layer_norm(NT, x_enc[:], D, eps5)
                xT2 = transpose_act(x_enc, D, "h1T")
                hmid = en_feat.tile([NT, D], F32, tag="h1")
                linear_nt(xT2, mp1T, 2, mp1b_rep, hmid, act=AF.Gelu)
                hT = transpose_act(hmid, D, "aoT")
                y_sb = en_feat.tile([NT, D], F32, tag="ao2")
                linear_nt(hT, mp2T, 2, mp2b_rep, y_sb)
                nc.sync.dma_start(y_out[:, :], y_sb)

    nc.compile()
    return nc


# ---------------------------------------------------------------- host glue
def _prep_arrays(params):
    p = params
    a = {}

    def npf(x):
        return np.ascontiguousarray(np.asarray(x, dtype=np.float32))

    a["Wt_vl"] = npf(p["vl_proj_w"]).T.copy()
    a["pb_vl"] = npf(p["vl_proj_b"]).reshape(1, D)
    a["Wt_r"] = npf(p["r_proj_w"]).T.copy()
    a["pb_r"] = npf(p["r_proj_b"]).reshape(1, D)
    a["WtC"] = npf(p["cond_obs_w"]).T.copy()
    a["pbC"] = npf(p["cond_obs_b"]).reshape(1, D)

    for pref in ("vl", "r"):
        in_w = npf(p[pref + "_in_w"])
        in_b = npf(p[pref + "_in_b"])
        wq, wk, wv = in_w[:D], in_w[D:2 * D], in_w[2 * D:]
        bq, bv = in_b[:D], in_b[2 * D:]
        q = npf(p[pref + "_pool_query"]).reshape(D)
        qn_w = npf(p[pref + "_qn_w"])
        kn_w = npf(p[pref + "_kn_w"])
        qh = q.reshape(H, HD)
        qn = (qh / np.sqrt((qh ** 2).mean(-1, keepdims=True) + 1e-6) * qn_w
              ).reshape(D)
        qproj = wq @ qn + bq
        U = np.zeros((H, D), np.float32)
        knw_t = np.tile(kn_w, H)
        for h in range(H):
            u0 = wk[h * HD:(h + 1) * HD].T @ qproj[h * HD:(h + 1) * HD]
            U[h] = u0 * knw_t / np.sqrt(HD)
        a["UT_" + pref] = U.T.copy()
        a["wvT_" + pref] = wv.T.copy()
        a["bv_" + pref] = bv.reshape(1, D)
        a["WoT_" + pref] = npf(p[pref + "_out_w"]).T.copy()
        a["ob_" + pref] = npf(p[pref + "_out_b"]).reshape(1, D)

    for i, L in enumerate(p["layers"]):
        a[f"L{i}_WinT"] = npf(L["in_w"]).T.copy()
        a[f"L{i}_inb"] = npf(L["in_b"]).reshape(1, 3 * D)
        a[f"L{i}_WoutT"] = npf(L["out_w"]).T.copy()
        a[f"L{i}_outb"] = npf(L["out_b"]).reshape(1, D)
        a[f"L{i}_W1T"] = npf(L["lin1_w"]).T.copy()
        a[f"L{i}_b1"] = npf(L["lin1_b"]).reshape(1, DFF)
        a[f"L{i}_W2T"] = npf(L["lin2_w"]).T.copy()
        a[f"L{i}_b2"] = npf(L["lin2_b"]).reshape(1, D)
        for nm in ("ln1_w", "ln2_w"):
            assert np.all(np.asarray(L[nm]) == 1.0), "non-trivial ln weight"
        for nm in ("ln1_b", "ln2_b"):
            assert np.all(np.asarray(L[nm]) == 0.0), "non-trivial ln bias"

    for nm in ("vl_ln_w", "r_ln_w", "pre_ln_w", "mem_ln_w"):
        assert np.all(np.asarray(p[nm]) == 1.0), "non-trivial ln weight"
    for nm in ("vl_ln_b", "r_ln_b", "pre_ln_b", "mem_ln_b"):
        assert np.all(np.asarray(p[nm]) == 0.0), "non-trivial ln bias"
    assert np.all(np.asarray(p["cond_pos_emb"]) == 0.0), "non-trivial pos emb"

    a["mp1T"] = npf(p["mp1_w"]).T.copy()
    a["mp1b"] = npf(p["mp1_b"]).reshape(1, D)
    a["mp2T"] = npf(p["mp2_w"]).T.copy()
    a["mp2b"] = npf(p["mp2_b"]).reshape(1, D)

    half = D // 2
    scale = np.log(10000.0) / (half - 1)
    a["freqs"] = np.exp(np.arange(half, dtype=np.float32) * -scale
                        ).reshape(1, half).astype(np.float32)

    mask = np.full((NT, NT), -1e9, np.float32)
    for b in range(BL):
        mask[b * T:(b + 1) * T, b * T:(b + 1) * T] = 0.0
    a["enc_mask"] = mask
    return a


def kernel(timesteps, cond, vl_features, reasoning_features, params):
    timesteps = np.asarray(timesteps, np.float32)
    cond = np.asarray(cond, np.float32)
    vl = np.asarray(vl_features, np.float32)
    rs = np.asarray(reasoning_features, np.float32)

    if "nc" not in _CACHE:
        _CACHE["nc"] = build_nc()
    nc = _CACHE["nc"]

    shared = _prep_arrays(params)
    in_maps = []
    for c in range(NCORES):
        bs = slice(c * BL, (c + 1) * BL)
        m = dict(shared)
        m["x_vl"] = np.ascontiguousarray(vl[bs].reshape(BL * S, KVL))
        m["x_r"] = np.ascontiguousarray(rs[bs].reshape(BL * S, KVL))
        m["x_cond"] = np.ascontiguousarray(cond[bs].reshape(BL * TO, 512))
        m["tvec"] = np.ascontiguousarray(timesteps[bs].reshape(BL, 1))
        in_maps.append(m)

    res = run_bass_kernel_spmd(nc, in_maps, core_ids=list(range(NCORES)))
    _CACHE["last_res"] = res
    out = np.concatenate([r["y"].reshape(BL, T, D) for r in res.results], axis=0)
    return out


# revision 22
# speedup vs baseline: 37.8162x; 1.0294x over previous
"""Trainium2 Bass kernel for nn_CustomEncoderBlock (dense transformer encoder).

Sharding: pure data parallel over batch (64) across 8 NeuronCores, params
replicated. Token-major activations; PE-transposes feed f32r matmuls.
Pool attention is algebraically folded: since q_len==1 and the pool query is
input-independent, the packed k-projection collapses into U @ rms(feat) and
the v-projection is applied after pooling (per-head weighted feature sums).
Encoder attention runs over all 88 packed tokens with a block-diagonal mask.
"""
import numpy as np

import concourse.bass as bass
import concourse.tile as tile
import concourse.mybir as mybir
from concourse import bacc
from concourse.bass_utils import run_bass_kernel_spmd
from concourse.masks import make_identity

F32 = mybir.dt.float32
F32R = mybir.dt.float32r
I32 = mybir.dt.int32
ADD = mybir.AluOpType.add
SUB = mybir.AluOpType.subtract
MUL = mybir.AluOpType.mult
MAX = mybir.AluOpType.max
AX = mybir.AxisListType.X
AF = mybir.ActivationFunctionType

NCORES = 8
B, TO, S, KVL = 64, 8, 512, 1536
D, H, HD = 1024, 16, 64
BL = B // NCORES
T = 3 + TO
NT = BL * T                      # 88 tokens per core
DFF = 4 * D
NL = 4
P = 128
KO = KVL // P                    # 12
DO = D // P                      # 8
TWO_PI = float(2 * np.pi)
HALF_PI = float(np.pi / 2)

_CACHE = {}


def build_nc():
    nc = bacc.Bacc(None, target_bir_lowering=False)

    def din(name, shape, dtype=F32):
        return nc.dram_tensor(name, list(shape), dtype, kind="ExternalInput")

    x_vl = din("x_vl", (BL * S, KVL))
    x_r = din("x_r", (BL * S, KVL))
    x_cond = din("x_cond", (BL * TO, 512))
    tvec = din("tvec", (BL, 1))
    freqs = din("freqs", (1, 512))
    enc_mask = din("enc_mask", (NT, NT))

    per_pref = {}
    for pref in ("vl", "r"):
        per_pref[pref] = dict(
            Wt=din(f"Wt_{pref}", (KVL, D), F32R),
            pb=din(f"pb_{pref}", (1, D)),
            UT=din(f"UT_{pref}", (D, H), F32R),
            wvT=din(f"wvT_{pref}", (D, D), F32R),
            bv=din(f"bv_{pref}", (1, D)),
            WoT=din(f"WoT_{pref}", (D, D), F32R),
            ob=din(f"ob_{pref}", (1, D)),
        )
    WtC = din("WtC", (512, D), F32R)
    pbC = din("pbC", (1, D))

    layers = []
    for i in range(NL):
        layers.append(dict(
            WinT=din(f"L{i}_WinT", (D, 3 * D), F32R),
            inb=din(f"L{i}_inb", (1, 3 * D)),
            WoutT=din(f"L{i}_WoutT", (D, D), F32R),
            outb=din(f"L{i}_outb", (1, D)),
            W1T=din(f"L{i}_W1T", (D, DFF), F32R),
            b1=din(f"L{i}_b1", (1, DFF)),
            W2T=din(f"L{i}_W2T", (DFF, D), F32R),
            b2=din(f"L{i}_b2", (1, D)),
        ))
    mp1T = din("mp1T", (D, D), F32R)
    mp1b = din("mp1b", (1, D))
    mp2T = din("mp2T", (D, D), F32R)
    mp2b = din("mp2b", (1, D))

    y_out = nc.dram_tensor("y", [NT, D], F32, kind="ExternalOutput")

    with tile.TileContext(nc) as tc:
        with tc.tile_pool(name="const", bufs=1) as const, \
             tc.tile_pool(name="sb_keep", bufs=1) as sb_keep, \
             tc.tile_pool(name="sb_small", bufs=1) as sb_small, \
             tc.tile_pool(name="sb_scr", bufs=1) as sb_scr, \
             tc.tile_pool(name="sb_stream", bufs=2) as sb_stream, \
             tc.tile_pool(name="ps_mm", bufs=2, space="PSUM") as ps_mm, \
             tc.tile_pool(name="ps_t", bufs=4, space="PSUM") as ps_t, \
             tc.tile_pool(name="ps_s", bufs=2, space="PSUM") as ps_s, \
             tc.tile_pool(name="dram", bufs=1, space="DRAM") as dram:

            ident = const.tile([P, P], F32)
            make_identity(nc, ident)
            eps5 = const.tile([P, 1], F32)
            nc.vector.memset(eps5, 1e-5)
            eps6 = const.tile([P, 1], F32)
            nc.vector.memset(eps6, 1e-6)

            def t_psum():
                return ps_t.tile([P, 512], F32, tag="t_ps", name="t_ps")

            def rep_vec(dram_vec, n, tag, pool=None, col0=0):
                """[1,n] slice of a DRAM vector -> [128,n] broadcast f32 tile."""
                rep = (pool or sb_keep).tile([P, n], F32, tag="rep_" + tag,
                                             name="rep_" + tag)
                src = dram_vec[:, col0:col0 + n]
                bc = bass.AP(tensor=src.tensor, offset=src.offset,
                             ap=[[0, P], [1, n]])
                nc.gpsimd.dma_start(out=rep, in_=bc)
                return rep

            def transpose_into(src_fn, dst, dst_j, rows, kblocks, dcol=None,
                               scale=None):
                for kb in range(kblocks):
                    pt = t_psum()
                    nc.tensor.transpose(pt[:, :rows], src_fn(kb),
                                        ident[:rows, :rows])
                    col = kb if dcol is None else dcol
                    dsl = dst[:, col, dst_j * rows:(dst_j + 1) * rows]
                    if scale is None:
                        # alternate eviction engine to decouple PE from DVE
                        if kb % 2 == 0:
                            nc.vector.tensor_copy(out=dsl, in_=pt[:, :rows])
                        else:
                            nc.scalar.activation(dsl, pt[:, :rows], AF.Copy)
                    else:
                        nc.vector.tensor_scalar(out=dsl, in0=pt[:, :rows],
                                                scalar1=scale, scalar2=None,
                                                op0=MUL)

            def layer_norm(rows, x_ap, width, eps_tile):
                ssum = sb_small.tile([P, 1], F32, tag="ln_sum", name="ln_sum")
                nc.vector.tensor_reduce(out=ssum[:rows], in_=x_ap, axis=AX, op=ADD)
                sq = sb_scr.tile([P, 1024], F32, tag="sq1024", name="sq")
                ssq = sb_small.tile([P, 1], F32, tag="ln_ssq", name="ln_ssq")
                nc.scalar.activation(sq[:rows, :width], x_ap, AF.Square,
                                     accum_out=ssq[:rows])
                nmean = sb_small.tile([P, 1], F32, tag="ln_nm", name="ln_nm")
                nc.vector.tensor_scalar(out=nmean[:rows], in0=ssum[:rows],
                                        scalar1=-1.0 / width, scalar2=None, op0=MUL)
                m2 = sb_small.tile([P, 1], F32, tag="ln_m2", name="ln_m2")
                nc.vector.tensor_tensor(out=m2[:rows], in0=nmean[:rows],
                                        in1=nmean[:rows], op=MUL)
                var = sb_small.tile([P, 1], F32, tag="ln_var", name="ln_var")
                nc.vector.tensor_scalar(out=var[:rows], in0=ssq[:rows],
                                        scalar1=1.0 / width, scalar2=m2[:rows],
                                        op0=MUL, op1=SUB)
                nc.scalar.activation(var[:rows], var[:rows], AF.Sqrt,
                                     bias=eps_tile[:rows])
                nc.vector.reciprocal(out=var[:rows], in_=var[:rows])
                nc.vector.tensor_scalar(out=x_ap, in0=x_ap, scalar1=nmean[:rows],
                                        scalar2=var[:rows], op0=ADD, op1=MUL)

            stream_pool = [sb_stream]

            def stream_unit(wdram, k0, n0, tag="wunit"):
                u = stream_pool[0].tile([P, DO, 512], F32R, tag=tag, name=tag)
                nc.sync.dma_start(
                    u, wdram[k0:k0 + D, n0:n0 + 512]
                    .rearrange("(c p) n -> p c n", p=P))
                return u

            # ======== t_emb ========
            te = sb_keep.tile([BL, D], F32, tag="te")
            with tc.tile_pool(name="temb", bufs=1) as temb:
                fr_rep = temb.tile([BL, 512], F32, tag="fr_rep")
                fsrc = freqs[:, :]
                nc.gpsimd.dma_start(out=fr_rep, in_=bass.AP(
                    tensor=fsrc.tensor, offset=fsrc.offset, ap=[[0, BL], [1, 512]]))
                t_sb = temb.tile([BL, 1], F32, tag="tvec")
                nc.sync.dma_start(t_sb, tvec[:, :])
                ang = temb.tile([BL, 512], F32, tag="ang")
                nc.vector.tensor_scalar_mul(ang, fr_rep, t_sb)
                for half, shift in ((0, 0.0), (1, HALF_PI)):
                    a2 = temb.tile([BL, 512], F32, tag="a2")
                    nc.vector.tensor_scalar(out=a2, in0=ang, scalar1=shift,
                                            scalar2=1.0 / TWO_PI, op0=ADD, op1=MUL)
                    mi = temb.tile([BL, 512], I32, tag="mi")
                    nc.vector.tensor_copy(out=mi, in_=a2)
                    mf = temb.tile([BL, 512], F32, tag="mf")
                    nc.vector.tensor_copy(out=mf, in_=mi)
                    nc.vector.tensor_tensor(out=mf, in0=a2, in1=mf, op=SUB)
                    nc.vector.tensor_scalar(out=mf, in0=mf, scalar1=TWO_PI,
                                            scalar2=None, op0=MUL)
                    nc.scalar.activation(te[:, half * 512:(half + 1) * 512], mf,
                                         AF.Sin)

            # ======== stage A ========
            pooled = {}
            with tc.tile_pool(name="sa_w", bufs=1) as sa_w, \
                 tc.tile_pool(name="sa_keep", bufs=1) as sa_keep, \
                 tc.tile_pool(name="sa_feat", bufs=1) as sa_feat, \
                 tc.tile_pool(name="sa_x", bufs=2) as sa_x:

                # ---- cond projection ----
                cond_e = sb_keep.tile([BL * TO, D], F32, tag="cond_e")
                with tc.tile_pool(name="sa_cond", bufs=1) as sa_cond:
                    WtC_sb = sa_cond.tile([P, 4, D], F32R, tag="WtC")
                    nc.sync.dma_start(WtC_sb,
                                      WtC[:, :].rearrange("(c p) n -> p c n", p=P))
                    pbC_rep = rep_vec(pbC, D, "pbC", pool=sa_cond)
                    xc = sa_x.tile([BL * TO, 512], F32, tag="xc")
                    nc.sync.dma_start(xc, x_cond[:, :])
                    condT = sa_cond.tile([P, 4, BL * TO], F32R, tag="condT")
                    transpose_into(lambda kb: xc[:, kb * P:(kb + 1) * P], condT,
                                   0, BL * TO, 4)
                    for n in range(2):
                        pm = ps_mm.tile([P, 512], F32, tag="mm_ps", name="pm")
                        for j in range(4):
                            nc.tensor.matmul(
                                pm[:BL * TO], lhsT=condT[:, j, :],
                                rhs=WtC_sb[:, j, n * 512:(n + 1) * 512],
                                start=(j == 0), stop=(j == 3))
                        nc.vector.tensor_tensor(
                            out=cond_e[:, n * 512:(n + 1) * 512],
                            in0=pm[:BL * TO],
                            in1=pbC_rep[:BL * TO, n * 512:(n + 1) * 512], op=ADD)

                # ---- per-modality projection + pool ----
                for pref, xin in (("vl", x_vl), ("r", x_r)):
                    pw = per_pref[pref]
                    W_sb = sa_w.tile([P, KO, D], F32R, tag="Wbig")
                    nc.sync.dma_start(
                        W_sb, pw["Wt"][:, :].rearrange("(c p) n -> p c n", p=P))
                    pb_rep = rep_vec(pw["pb"], D, "pb", pool=sa_keep)
                    UT_sb = sa_w.tile([P, DO, H], F32R, tag="UT")
                    nc.sync.dma_start(
                        UT_sb, pw["UT"][:, :].rearrange("(c p) n -> p c n", p=P))
                    bv_rep = rep_vec(pw["bv"], D, "bv", pool=sa_keep)
                    ob_rep = rep_vec(pw["ob"], D, "ob", pool=sa_keep)

                    GT = sa_keep.tile([P, DO, H, BL], F32R, tag="GT")
                    xin_v = xin[:, :].rearrange("(b s) k -> b s k", s=S)

                    for b in range(BL):
                        feat = sa_feat.tile([P, S // P, D], F32R, tag="feat")
                        for i in range(S // P):
                            x_in = sa_x.tile([P, KVL], F32, tag="x_in")
                            nc.sync.dma_start(x_in, xin_v[b, i * P:(i + 1) * P, :])
                            xT_c = sa_x.tile([P, KO, P], F32R, tag="xT_c")
                            transpose_into(
                                lambda kb: x_in[:, kb * P:(kb + 1) * P],
                                xT_c, 0, P, KO)
                            for n in range(2):
                                pm = ps_mm.tile([P, 512], F32, tag="mm_ps",
                                                name="pm")
                                for j in range(KO):
                                    nc.tensor.matmul(
                                        pm, lhsT=xT_c[:, j, :],
                                        rhs=W_sb[:, j, n * 512:(n + 1) * 512],
                                        start=(j == 0), stop=(j == KO - 1))
                                nc.vector.tensor_tensor(
                                    out=feat[:, i, n * 512:(n + 1) * 512],
                                    in0=pm, in1=pb_rep[:, n * 512:(n + 1) * 512],
                                    op=ADD)
                            layer_norm(P, feat[:, i, :], D, eps5)

                        # per-head rms rstd [128, 4, 16]
                        rstd = sb_small.tile([P, S // P, H], F32, tag="rstd")
                        for i in range(S // P):
                            sq = sb_scr.tile([P, 1024], F32, tag="sq1024",
                                             name="sq")
                            nc.scalar.activation(sq, feat[:, i, :], AF.Square)
                            nc.vector.tensor_reduce(
                                out=rstd[:, i, :],
                                in_=sq[:].rearrange("p (h d) -> p h d", d=HD),
                                axis=AX, op=ADD)
                        rsf = rstd[:].rearrange("p a b -> p (a b)")
                        nc.scalar.activation(rsf, rsf, AF.Sqrt, bias=eps6,
                                             scale=1.0 / HD)
                        nc.vector.reciprocal(out=rsf, in_=rsf)

                        # kn per s-chunk -> knT_i [128, 8, 128] f32r -> scores
                        psc = ps_s.tile([P, 512], F32, tag="s_ps", name="psc")
                        for i in range(S // P):
                            knT_i = sa_x.tile([P, DO, P], F32R, tag="knT_i")
                            for m in range(DO):
                                knb = sb_scr.tile([P, P], F32, tag="knb",
                                                  name="knb")
                                nc.vector.tensor_tensor(
                                    out=knb[:].rearrange("p (a b) -> p a b", b=HD),
                                    in0=feat[:, i, m * P:(m + 1) * P].rearrange(
                                        "p (a b) -> p a b", b=HD),
                                    in1=rstd[:, i, 2 * m:2 * m + 2, None]
                                    .to_broadcast((P, 2, HD)),
                                    op=MUL)
                                pt2 = t_psum()
                                nc.tensor.transpose(pt2[:, :P], knb, ident)
                                nc.vector.tensor_copy(out=knT_i[:, m, :],
                                                      in_=pt2[:, :P])
                            for m in range(DO):
                                nc.tensor.matmul(
                                    psc[:H, i * P:(i + 1) * P],
                                    lhsT=UT_sb[:, m, :], rhs=knT_i[:, m, :],
                                    start=(m == 0), stop=(m == DO - 1))
                        nmax = sb_small.tile([H, 1], F32, tag="nmax")
                        nc.vector.tensor_reduce(out=nmax, in_=psc[:H], axis=AX,
                                                op=MAX, negate=True)
                        attn = sb_small.tile([H, S], F32, tag="attn")
                        den = sb_small.tile([H, 1], F32, tag="den")
                        nc.scalar.activation(attn, psc[:H], AF.Exp, bias=nmax,
                                             accum_out=den)
                        nc.vector.reciprocal(out=den, in_=den)
                        nc.vector.tensor_scalar_mul(attn, attn, den)
                        attnT = sb_small.tile([P, S // P, H], F32R, tag="attnT")
                        transpose_into(lambda kb: attn[:, kb * P:(kb + 1) * P],
                                       attnT, 0, H, S // P)
                        for m in range(DO):
                            pg = ps_s.tile([P, 512], F32, tag="s_ps", name="pg")
                            for i in range(S // P):
                                nc.tensor.matmul(
                                    pg[:, :H],
                                    lhsT=feat[:, i, m * P:(m + 1) * P],
                                    rhs=attnT[:, i, :],
                                    start=(i == 0), stop=(i == S // P - 1))
                            nc.vector.tensor_copy(out=GT[:, m, :, b],
                                                  in_=pg[:, :H])

                    # ---- apply wv per head, then out-proj ----
                    O_sb = sb_small.tile([BL, D], F32, tag="O_sb")
                    for half in range(2):
                        po = ps_mm.tile([P, 512], F32, tag="mm_ps", name="po")
                        wu = stream_unit(pw["wvT"], 0, half * 512)
                        for h8 in range(DO):
                            h = half * DO + h8
                            for k in range(DO):
                                nc.tensor.matmul(
                                    po[:BL, h8 * HD:(h8 + 1) * HD],
                                    lhsT=GT[:, k, h, :],
                                    rhs=wu[:, k, h8 * HD:(h8 + 1) * HD],
                                    start=(k == 0), stop=(k == DO - 1))
                        nc.vector.tensor_tensor(
                            out=O_sb[:, half * 512:(half + 1) * 512],
                            in0=po[:BL],
                            in1=bv_rep[:BL, half * 512:(half + 1) * 512], op=ADD)
                    OT = sb_small.tile([P, DO, BL], F32R, tag="OT")
                    transpose_into(lambda kb: O_sb[:, kb * P:(kb + 1) * P], OT,
                                   0, BL, DO)
                    pooled_sb = sb_keep.tile([BL, D], F32, tag="pooled_" + pref)
                    for n in range(2):
                        wu = stream_unit(pw["WoT"], 0, n * 512)
                        pm = ps_mm.tile([P, 512], F32, tag="mm_ps", name="pm")
                        for k in range(DO):
                            nc.tensor.matmul(pm[:BL], lhsT=OT[:, k, :],
                                             rhs=wu[:, k, :],
                                             start=(k == 0), stop=(k == DO - 1))
                        nc.vector.tensor_tensor(
                            out=pooled_sb[:, n * 512:(n + 1) * 512], in0=pm[:BL],
                            in1=ob_rep[:BL, n * 512:(n + 1) * 512], op=ADD)
                    pooled[pref] = pooled_sb

            # ======== assemble encoder input ========
            stage = dram.tile([NT, D], F32)
            st_v = stage[:].rearrange("(b t) d -> b t d", t=T)
            nc.sync.dma_start(st_v[:, 0, :], te)
            nc.sync.dma_start(st_v[:, 1, :], pooled["vl"])
            nc.sync.dma_start(st_v[:, 2, :], pooled["r"])
            nc.sync.dma_start(st_v[:, 3:, :], cond_e)
            x_enc = sb_keep.tile([NT, D], F32, tag="x_enc")
            nc.sync.dma_start(x_enc, stage[:])

            mask_sb = const.tile([NT, NT], F32)
            nc.sync.dma_start(mask_sb, enc_mask[:, :])

            layer_norm(NT, x_enc[:], D, eps5)

            # ======== encoder layers + head ========
            with tc.tile_pool(name="en_feat", bufs=1) as en_feat, \
                 tc.tile_pool(name="en_xt", bufs=1) as en_xt, \
                 tc.tile_pool(name="en_stream", bufs=3) as en_stream:
                stream_pool[0] = en_stream

                def transpose_act(src, width, tag):
                    dst = en_xt.tile([P, width // P, NT], F32R, tag=tag,
                                     name=tag)
                    transpose_into(lambda kb: src[:, kb * P:(kb + 1) * P], dst,
                                   0, NT, width // P)
                    return dst

                def linear_nt(xT_t, wdram, nslices, brep, out_tile, act=None):
                    for n in range(nslices):
                        pm = ps_mm.tile([P, 512], F32, tag="mm_ps", name="pm")
                        wu = stream_unit(wdram, 0, n * 512)
                        for k in range(DO):
                            nc.tensor.matmul(
                                pm[:NT], lhsT=xT_t[:, k, :], rhs=wu[:, k, :],
                                start=(k == 0), stop=(k == DO - 1))
                        osl = out_tile[:, n * 512:(n + 1) * 512]
                        bsl = brep[:NT, n * 512:(n + 1) * 512]
                        if act is None:
                            nc.vector.tensor_tensor(out=osl, in0=pm[:NT], in1=bsl,
                                                    op=ADD)
                        else:
                            nc.vector.tensor_tensor(out=pm[:NT], in0=pm[:NT],
                                                    in1=bsl, op=ADD)
                            nc.scalar.activation(osl, pm[:NT], act)

                for li, L in enumerate(layers):
                    inb_rep = rep_vec(L["inb"], 3 * D, "inb", pool=en_feat)
                    outb_rep = rep_vec(L["outb"], D, "outb", pool=en_feat)
                    b2_rep = rep_vec(L["b2"], D, "b2", pool=en_feat)

                    h1 = en_feat.tile([NT, D], F32, tag="h1")
                    nc.vector.tensor_copy(out=h1, in_=x_enc)
                    layer_norm(NT, h1[:], D, eps5)
                    h1T = transpose_act(h1, D, "h1T")
                    qkv = en_feat.tile([NT, 3 * D], F32, tag="qkv")
                    for n in range(6):
                        pm = ps_mm.tile([P, 512], F32, tag="mm_ps", name="pm")
                        wu = stream_unit(L["WinT"], 0, n * 512)
                        for k in range(DO):
                            nc.tensor.matmul(pm[:NT], lhsT=h1T[:, k, :],
                                             rhs=wu[:, k, :],
                                             start=(k == 0), stop=(k == DO - 1))
                        nc.vector.tensor_tensor(
                            out=qkv[:, n * 512:(n + 1) * 512], in0=pm[:NT],
                            in1=inb_rep[:NT, n * 512:(n + 1) * 512], op=ADD)

                    qkT = en_xt.tile([P, 2 * DO, NT], F32, tag="qkT")
                    for kb in range(2 * DO):
                        sc = 0.125 if kb < DO else None
                        transpose_into(
                            lambda _, kb=kb: qkv[:, kb * P:(kb + 1) * P], qkT,
                            0, NT, 1, dcol=kb, scale=sc)
                    S_sb = en_feat.tile([NT, H, NT], F32, tag="S_sb")
                    for h in range(H):
                        base = HD * (h % 2)
                        psc = ps_s.tile([P, 512], F32, tag="s_ps", name="psc")
                        nc.tensor.matmul(psc[:NT, :NT],
                                         lhsT=qkT[base:base + HD, h // 2, :],
                                         rhs=qkT[base:base + HD, DO + h // 2, :],
                                         start=True, stop=True)
                        nc.vector.tensor_tensor(out=S_sb[:, h, :],
                                                in0=psc[:NT, :NT], in1=mask_sb,
                                                op=ADD)
                    nmax = sb_small.tile([NT, H], F32, tag="nmax_e")
                    nc.vector.tensor_reduce(out=nmax, in_=S_sb, axis=AX, op=MAX,
                                            negate=True)
                    nc.vector.tensor_tensor(
                        out=S_sb, in0=S_sb,
                        in1=nmax[:, :, None].to_broadcast((NT, H, NT)), op=ADD)
                    sflat = S_sb[:].rearrange("p a b -> p (a b)")
                    nc.scalar.activation(sflat, sflat, AF.Exp)
                    den = sb_small.tile([NT, H], F32, tag="den_e")
                    nc.vector.tensor_reduce(out=den, in_=S_sb, axis=AX, op=ADD)
                    nc.vector.reciprocal(out=den, in_=den)
                    nc.vector.tensor_tensor(
                        out=S_sb, in0=S_sb,
                        in1=den[:, :, None].to_broadcast((NT, H, NT)), op=MUL)

                    pav0 = ps_s.tile([P, 512], F32, tag="s_ps", name="pav0")
                    pav1 = ps_s.tile([P, 512], F32, tag="s_ps", name="pav1")
                    pavs = (pav0, pav1)
                    for h in range(H):
                        pt2 = t_psum()
                        nc.tensor.transpose(pt2[:NT, :NT], S_sb[:, h, :],
                                            ident[:NT, :NT])
                        aT = sb_scr.tile([NT, NT], F32, tag="aT", name="aT")
                        nc.vector.tensor_copy(out=aT, in_=pt2[:NT, :NT])
                        nc.tensor.matmul(
                            pavs[h // DO][:NT, (h % DO) * HD:(h % DO + 1) * HD],
                            lhsT=aT,
                            rhs=qkv[:, 2 * D + h * HD:2 * D + (h + 1) * HD],
                            start=True, stop=True)
                    ao = en_feat.tile([NT, D], F32, tag="ao")
                    for i in range(2):
                        nc.vector.tensor_copy(out=ao[:, i * 512:(i + 1) * 512],
                                              in_=pavs[i][:NT])
                    aoT = transpose_act(ao, D, "aoT")
                    attn_proj = en_feat.tile([NT, D], F32, tag="ao2")
                    linear_nt(aoT, L["WoutT"], 2, outb_rep, attn_proj)
                    nc.vector.tensor_tensor(out=x_enc, in0=x_enc, in1=attn_proj,
                                            op=ADD)

                    # FFN in two 2048-halves to bound SBUF
                    h2 = en_feat.tile([NT, D], F32, tag="h1")
                    nc.vector.tensor_copy(out=h2, in_=x_enc)
                    layer_norm(NT, h2[:], D, eps5)
                    h2T = transpose_act(h2, D, "h1T")
                    pm0 = ps_mm.tile([P, 512], F32, tag="mm_ps", name="pm0")
                    pm1 = ps_mm.tile([P, 512], F32, tag="mm_ps", name="pm1")
                    pmn = (pm0, pm1)
                    for half in range(2):
                        b1h = rep_vec(L["b1"], 2048, "b1h", pool=en_feat,
                                      col0=half * 2048)
                        g_half = en_feat.tile([NT, 2048], F32, tag="g_half")
                        for u in range(4):
                            pmg = ps_s.tile([P, 512], F32, tag="s_ps", name="pmg")
                            wu = stream_unit(L["W1T"], 0, half * 2048 + u * 512)
                            for k in range(DO):
                                nc.tensor.matmul(pmg[:NT], lhsT=h2T[:, k, :],
                                                 rhs=wu[:, k, :],
                                                 start=(k == 0), stop=(k == DO - 1))
                            nc.vector.tensor_tensor(
                                out=pmg[:NT], in0=pmg[:NT],
                                in1=b1h[:NT, u * 512:(u + 1) * 512], op=ADD)
                            nc.scalar.activation(
                                g_half[:, u * 512:(u + 1) * 512], pmg[:NT],
                                AF.Gelu)
                        gTh = en_xt.tile([P, 16, NT], F32R, tag="gTh",
                                         name="gTh")
                        transpose_into(
                            lambda kb: g_half[:, kb * P:(kb + 1) * P], gTh, 0,
                            NT, 16)
                        for kg in range(2):
                            for n in range(2):
                                wu2 = stream_unit(L["W2T"],
                                                  (half * 2 + kg) * D, n * 512)
                                for k in range(DO):
                                    nc.tensor.matmul(
                                        pmn[n][:NT],
                                        lhsT=gTh[:, kg * DO + k, :],
                                        rhs=wu2[:, k, :],
                                        start=(half == 0 and kg == 0 and k == 0),
                                        stop=(half == 1 and kg == 1 and
                                              k == DO - 1))
                    for n in range(2):
                        nc.vector.tensor_tensor(
                            out=pmn[n][:NT], in0=pmn[n][:NT],
                            in1=b2_rep[:NT, n * 512:(n + 1) * 512], op=ADD)
                        nc.vector.tensor_tensor(
                            out=x_enc[:, n * 512:(n + 1) * 512],
                            in0=x_enc[:, n * 512:(n + 1) * 512],
                            in1=pmn[n][:NT], op=ADD)

                # ---- head ----
                mp1b_rep = rep_vec(mp1b, D, "outb", pool=en_feat)
                mp2b_rep = rep_vec(mp2b, D, "b2", pool=en_feat)
                layer_norm(NT, x_enc[:], D, eps5)
                xT2 = transpose_act(x_enc, D, "h1T")
                hmid = en_feat.tile([NT, D], F32, tag="h1")
                linear_nt(xT2, mp1T, 2, mp1b_rep, hmid, act=AF.Gelu)
                hT = transpose_act(hmid, D, "aoT")
                y_sb = en_feat.tile([NT, D], F32, tag="ao2")
                linear_nt(hT, mp2T, 2, mp2b_rep, y_sb)
                nc.sync.dma_start(y_out[:, :], y_sb)

    nc.compile()
    return nc


# ---------------------------------------------------------------- host glue
def _prep_arrays(params):
    p = params
    a = {}

    def npf(x):
        return np.ascontiguousarray(np.asarray(x, dtype=np.float32))

    a["Wt_vl"] = npf(p["vl_proj_w"]).T.copy()
    a["pb_vl"] = npf(p["vl_proj_b"]).reshape(1, D)
    a["Wt_r"] = npf(p["r_proj_w"]).T.copy()
    a["pb_r"] = npf(p["r_proj_b"]).reshape(1, D)
    a["WtC"] = npf(p["cond_obs_w"]).T.copy()
    a["pbC"] = npf(p["cond_obs_b"]).reshape(1, D)

    for pref in ("vl", "r"):
        in_w = npf(p[pref + "_in_w"])
        in_b = npf(p[pref + "_in_b"])
        wq, wk, wv = in_w[:D], in_w[D:2 * D], in_w[2 * D:]
        bq, bv = in_b[:D], in_b[2 * D:]
        q = npf(p[pref + "_pool_query"]).reshape(D)
        qn_w = npf(p[pref + "_qn_w"])
        kn_w = npf(p[pref + "_kn_w"])
        qh = q.reshape(H, HD)
        qn = (qh / np.sqrt((qh ** 2).mean(-1, keepdims=True) + 1e-6) * qn_w
              ).reshape(D)
        qproj = wq @ qn + bq
        U = np.zeros((H, D), np.float32)
        knw_t = np.tile(kn_w, H)
        for h in range(H):
            u0 = wk[h * HD:(h + 1) * HD].T @ qproj[h * HD:(h + 1) * HD]
            U[h] = u0 * knw_t / np.sqrt(HD)
        a["UT_" + pref] = U.T.copy()
        a["wvT_" + pref] = wv.T.copy()
        a["bv_" + pref] = bv.reshape(1, D)
        a["WoT_" + pref] = npf(p[pref + "_out_w"]).T.copy()
        a["ob_" + pref] = npf(p[pref + "_out_b"]).reshape(1, D)

    for i, L in enumerate(p["layers"]):
        a[f"L{i}_WinT"] = npf(L["in_w"]).T.copy()
        a[f"L{i}_inb"] = npf(L["in_b"]).reshape(1, 3 * D)
        a[f"L{i}_WoutT"] = npf(L["out_w"]).T.copy()
        a[f"L{i}_outb"] = npf(L["out_b"]).reshape(1, D)
        a[f"L{i}_W1T"] = npf(L["lin1_w"]).T.copy()
        a[f"L{i}_b1"] = npf(L["lin1_b"]).reshape(1, DFF)
        a[f"L{i}_W2T"] = npf(L["lin2_w"]).T.copy()
        a[f"L{i}_b2"] = npf(L["lin2_b"]).reshape(1, D)
        for nm in ("ln1_w", "ln2_w"):
            assert np.all(np.asarray(L[nm]) == 1.0), "non-trivial ln weight"
        for nm in ("ln1_b", "ln2_b"):
            assert np.all(np.asarray(L[nm]) == 0.0), "non-trivial ln bias"

    for nm in ("vl_ln_w", "r_ln_w", "pre_ln_w", "mem_ln_w"):
        assert np.all(np.asarray(p[nm]) == 1.0), "non-trivial ln weight"
    for nm in ("vl_ln_b", "r_ln_b", "pre_ln_b", "mem_ln_b"):
        assert np.all(np.asarray(p[nm]) == 0.0), "non-trivial ln bias"
    assert np.all(np.asarray(p["cond_pos_emb"]) == 0.0), "non-trivial pos emb"

    a["mp1T"] = npf(p["mp1_w"]).T.copy()
    a["mp1b"] = npf(p["mp1_b"]).reshape(1, D)
    a["mp2T"] = npf(p["mp2_w"]).T.copy()
    a["mp2b"] = npf(p["mp2_b"]).reshape(1, D)

    half = D // 2
    scale = np.log(10000.0) / (half - 1)
    a["freqs"] = np.exp(np.arange(half, dtype=np.float32) * -scale
                        ).reshape(1, half).astype(np.float32)

    mask = np.full((NT, NT), -1e9, np.float32)
    for b in range(BL):
        mask[b * T:(b + 1) * T, b * T:(b + 1) * T] = 0.0
    a["enc_mask"] = mask
    return a


def kernel(timesteps, cond, vl_features, reasoning_features, params):
    timesteps = np.asarray(timesteps, np.float32)
    cond = np.asarray(cond, np.float32)
    vl = np.asarray(vl_features, np.float32)
    rs = np.asarray(reasoning_features, np.float32)

    if "nc" not in _CACHE:
        _CACHE["nc"] = build_nc()
    nc = _CACHE["nc"]

    shared = _prep_arrays(params)
    in_maps = []
    for c in range(NCORES):
        bs = slice(c * BL, (c + 1) * BL)
        m = dict(shared)
        m["x_vl"] = np.ascontiguousarray(vl[bs].reshape(BL * S, KVL))
        m["x_r"] = np.ascontiguousarray(rs[bs].reshape(BL * S, KVL))
        m["x_cond"] = np.ascontiguousarray(cond[bs].reshape(BL * TO, 512))
        m["tvec"] = np.ascontiguousarray(timesteps[bs].reshape(BL, 1))
        in_maps.append(m)

    res = run_bass_kernel_spmd(nc, in_maps, core_ids=list(range(NCORES)))
    _CACHE["last_res"] = res
    out = np.concatenate([r["y"].reshape(BL, T, D) for r in res.results], axis=0)
    return out
